# revision 2
# baseline (speedup 1.0000x reference)
"""Trainium2 Bass kernel for nn_FCN8sAtOnceMultiGnn2 (gnn_message_passing).

Strategy (per 8 NeuronCores; sample s = core//2, node-half = core%2):
  The GNN messages only feed a per-(sample,channel) SE gate: m_r/m_i are
  consumed by a full mean over nodes, so per iteration we only need
    S[c] = sum_edges lrelu(P[r_e,c] - Q[q_e,c] + b_c)
  where P/Q are per-sample tables h @ W (h = gate-scaled pooled features).
  The final output is relu(g1*prod(gate)*rgb_pooled + g2*prod(1-gate)*ir_pooled).

  Per core: maxpool -> bf16 Gram -> top-16 via DVE max8/max_index/match_replace
  -> edge lists -> per iteration: scale weights by accumulated gates, compute
  combined tables T_r=[Wr1+Wr2 | Wi2], T_q=[Wr2 | Wi1+Wi2] applied to bf16
  features (+bias), write to DRAM, dma_gather rows at edge indices, reduce
  sum(d) and sum(|d|) per channel via ones-matmul on PE (lrelu = .505x+.495|x|),
  pairwise AllReduce partial sums, SE MLP -> gate. Host reassembles halves.
"""
import sys

sys.path.insert(0, "/opt/trn_rl_repo")

import numpy as np

_CACHE = {}

P = 128
C = 512          # channels
NT = 1024        # nodes per sample (32*32 after pool)
HN = 512         # nodes per core (half sample)
KNN = 16
E = HN * KNN     # 8192 edges per core per direction
ECH = 1024       # edges per gather chunk
NCHUNK = E // ECH
N_CORES = 8


def _build(iterations: int):
    import concourse.bacc as bacc
    import concourse.mybir as mybir
    import concourse.tile as tile

    dt = mybir.dt
    f32, bf16, i16, u16 = dt.float32, dt.bfloat16, dt.int16, dt.uint16
    AF = mybir.ActivationFunctionType
    OP = mybir.AluOpType

    nc = bacc.Bacc("TRN2", target_bir_lowering=False, debug=False,
                   num_devices=N_CORES)

    rgb_in = nc.dram_tensor("rgb", [C, 64, 64], f32, kind="ExternalInput")
    ir_in = nc.dram_tensor("ir", [C, 64, 64], f32, kind="ExternalInput")
    wrgb_in = nc.dram_tensor("wrgb", [2 * C, C], f32, kind="ExternalInput")
    wir_in = nc.dram_tensor("wir", [2 * C, C], f32, kind="ExternalInput")
    brgb_in = nc.dram_tensor("brgb", [1, C], f32, kind="ExternalInput")
    bir_in = nc.dram_tensor("bir", [1, C], f32, kind="ExternalInput")
    wse1_in = nc.dram_tensor("wse1", [2 * C, 32], f32, kind="ExternalInput")
    bse1_in = nc.dram_tensor("bse1", [1, 32], f32, kind="ExternalInput")
    wse2_in = nc.dram_tensor("wse2", [32, C], f32, kind="ExternalInput")
    bse2_in = nc.dram_tensor("bse2", [1, C], f32, kind="ExternalInput")
    g1_in = nc.dram_tensor("g1", [1, 1], f32, kind="ExternalInput")
    g2_in = nc.dram_tensor("g2", [1, 1], f32, kind="ExternalInput")
    out_t = nc.dram_tensor("out", [C, HN], f32, kind="ExternalOutput")

    MODS = ("r", "i")
    mod_in = {"r": rgb_in, "i": ir_in}

    with tile.TileContext(nc) as tc:
        with (
            tc.tile_pool(name="persist", bufs=1) as pp,
            tc.tile_pool(name="big", bufs=3) as bigp,
            tc.tile_pool(name="dram", bufs=1, space="DRAM") as dram,
        ):
            # ---------------- constants ----------------
            ones_bf = pp.tile([P, 1], bf16, tag="ones_bf")
            nc.vector.memset(ones_bf[:], 1.0)
            ones_row = pp.tile([1, P], f32, tag="ones_row")
            nc.vector.memset(ones_row[:], 1.0)

            # persistent per-(mod, chunk) tiles
            xb = {m: [pp.tile([P, NT], bf16, tag=f"xb_{m}{cc}", name=f"xb_{m}{cc}")
                      for cc in range(4)] for m in MODS}
            phalf = {m: [pp.tile([P, HN], f32, tag=f"ph_{m}{cc}", name=f"ph_{m}{cc}")
                         for cc in range(4)] for m in MODS}
            rn = {m: pp.tile([1, NT], f32, tag=f"rn_{m}", name=f"rn_{m}")
                  for m in MODS}
            rni2x = {m: pp.tile([P, 4], f32, tag=f"rni_{m}", name=f"rni_{m}")
                     for m in MODS}
            idx_mt = {m: [pp.tile([P, KNN], u16, tag=f"ix_{m}{t}", name=f"ix_{m}{t}")
                          for t in range(4)] for m in MODS}
            eidx = {m: pp.tile([P, E // 16], i16, tag=f"eix_{m}", name=f"eix_{m}")
                    for m in MODS}

            # ---------------- stage 1: load + pool + norms ----------------
            with (
                tc.tile_pool(name="s1", bufs=1) as s1,
                tc.tile_pool(name="ps_ss", bufs=1, space="PSUM") as ps_ss_p,
            ):
                ps_ss = {m: [ps_ss_p.tile([1, C], f32, space="PSUM",
                                          tag=f"ss{m}{h}", name=f"ss{m}{h}")
                             for h in range(2)] for m in MODS}
                for m in MODS:
                    for cc in range(4):
                        raw = s1.tile([P, 64, 64], f32, tag="raw", name="raw", bufs=2)
                        nc.sync.dma_start(raw[:], mod_in[m][cc * P:(cc + 1) * P])
                        h1 = s1.tile([P, 32, 64], f32, tag="h1", name="h1")
                        nc.vector.tensor_tensor(out=h1[:], in0=raw[:, 0::2, :],
                                                in1=raw[:, 1::2, :], op=OP.max)
                        pf = s1.tile([P, 32, 32], f32, tag="pf", name="pf", bufs=2)
                        nc.vector.tensor_tensor(out=pf[:], in0=h1[:, :, 0::2],
                                                in1=h1[:, :, 1::2], op=OP.max)
                        pff = pf.rearrange("p a b -> p (a b)")
                        nc.vector.tensor_copy(xb[m][cc][:], pff)
                        nc.vector.tensor_copy(phalf[m][cc][:], pff[:, 0:HN])
                        sq = s1.tile([P, NT], bf16, tag="sq", name="sq")
                        nc.vector.tensor_tensor(out=sq[:], in0=pff, in1=pff,
                                                op=OP.mult)
                        for h in range(2):
                            nc.tensor.matmul(ps_ss[m][h][:], ones_bf[:],
                                             sq[:, h * C:(h + 1) * C],
                                             start=(cc == 0), stop=(cc == 3))
                # norms: rn = 1/max(sqrt(ss), 1e-12); Drow = ss * rn^2
                for m in MODS:
                    srow = s1.tile([1, NT], f32, tag="srow", name="srow")
                    for h in range(2):
                        nc.scalar.activation(srow[:, h * C:(h + 1) * C],
                                             ps_ss[m][h][:], AF.Sqrt)
                    nc.vector.tensor_scalar_max(srow[:], srow[:], 1e-12)
                    nc.vector.reciprocal(rn[m][:], srow[:])
                    # Drow = ss * rn * rn  (reuse srow as scratch)
                    nc.vector.tensor_tensor(out=srow[:], in0=rn[m][:],
                                            in1=rn[m][:], op=OP.mult)
                    for h in range(2):
                        nc.vector.tensor_tensor(out=srow[:, h * C:(h + 1) * C],
                                                in0=srow[:, h * C:(h + 1) * C],
                                                in1=ps_ss[m][h][:], op=OP.mult)
                    drow_d = dram.tile([1, NT], f32, tag=f"drow_{m}",
                                       name=f"drow_{m}")
                    nc.sync.dma_start(drow_d[:], srow[:])
                    # rni2x: columns of rn (my nodes 0..511) * 2, via DRAM trip
                    rhd = dram.tile([1, HN], f32, tag=f"rhd_{m}", name=f"rhd_{m}")
                    nc.sync.dma_start(rhd[:], rn[m][:, 0:HN])
                    nc.sync.dma_start(
                        rni2x[m][:],
                        rhd[:].rearrange("one (c p) -> (one p) c", p=P))
                    nc.vector.tensor_scalar_mul(rni2x[m][:], rni2x[m][:], 2.0)
                    dram_drow = drow_d  # keep handle
                    rn_dram = {m: None}
                    # stash on dict for stage 2 use
                    if m == "r":
                        drow_r = drow_d
                    else:
                        drow_i = drow_d
            drow_d_map = {"r": drow_r, "i": drow_i}

            # ---------------- stage 2: Gram + nd + top16 ----------------
            with (
                tc.tile_pool(name="s2", bufs=2) as s2,
                tc.tile_pool(name="s2b", bufs=1) as s2b,
                tc.tile_pool(name="ps_g", bufs=2, space="PSUM") as ps_g_p,
                tc.tile_pool(name="ps_b", bufs=2, space="PSUM") as ps_b_p,
            ):
                B = {}
                Db = {}
                for m in MODS:
                    B[m] = s2b.tile([P, NT], f32, tag=f"B_{m}", name=f"B_{m}")
                    Db[m] = s2b.tile([P, NT], f32, tag=f"Db_{m}", name=f"Db_{m}")
                    for h in range(2):
                        psb = ps_b_p.tile([P, C], f32, space="PSUM", tag="psb",
                                          name="psb")
                        nc.tensor.matmul(psb[:], ones_row[:],
                                         rn[m][:, h * C:(h + 1) * C],
                                         start=True, stop=True)
                        nc.vector.tensor_copy(B[m][:, h * C:(h + 1) * C], psb[:])
                    # broadcast Drow from DRAM row: simple DMA with step-0 AP
                    # (read same row into all 128 partitions) is not expressible;
                    # use matmul broadcast via a staged row.
                    drow_sb = s2.tile([1, NT], f32, tag="drow_sb", name="drow_sb")
                    nc.sync.dma_start(drow_sb[:], drow_d_map[m][:])
                    for h in range(2):
                        psb = ps_b_p.tile([P, C], f32, space="PSUM", tag="psb",
                                          name="psb")
                        nc.tensor.matmul(psb[:], ones_row[:],
                                         drow_sb[:, h * C:(h + 1) * C],
                                         start=True, stop=True)
                        nc.vector.tensor_copy(Db[m][:, h * C:(h + 1) * C], psb[:])

                for m in MODS:
                    for t in range(4):
                        nd = s2.tile([P, NT], f32, tag="nd", name="nd")
                        for h in range(2):
                            psg = ps_g_p.tile([P, C], f32, space="PSUM",
                                              tag="psg", name="psg")
                            for k in range(4):
                                nc.tensor.matmul(
                                    psg[:],
                                    xb[m][k][:, t * P:(t + 1) * P],
                                    xb[m][k][:, h * C:(h + 1) * C],
                                    start=(k == 0), stop=(k == 3))
                            tmp = s2.tile([P, C], f32, tag="tmp", name="tmp")
                            nc.vector.tensor_tensor(
                                out=tmp[:], in0=psg[:],
                                in1=B[m][:, h * C:(h + 1) * C], op=OP.mult)
                            nc.vector.tensor_scalar(
                                tmp[:], tmp[:], rni2x[m][:, t:t + 1], None,
                                op0=OP.mult)
                            nc.vector.tensor_tensor(
                                out=nd[:, h * C:(h + 1) * C], in0=tmp[:],
                                in1=Db[m][:, h * C:(h + 1) * C], op=OP.subtract)
                        mx = s2.tile([P, 16], f32, tag="mx", name="mx")
                        nc.vector.max(out=mx[:, 0:8], in_=nd[:])
                        nc.vector.max_index(out=idx_mt[m][t][:, 0:8],
                                            in_max=mx[:, 0:8], in_values=nd[:])
                        nc.vector.match_replace(out=nd[:], in_to_replace=mx[:, 0:8],
                                                in_values=nd[:], imm_value=-1e30)
                        nc.vector.max(out=mx[:, 8:16], in_=nd[:])
                        nc.vector.max_index(out=idx_mt[m][t][:, 8:16],
                                            in_max=mx[:, 8:16], in_values=nd[:])

            # ---------------- stage 3: edge index lists ----------------
            for m in MODS:
                exd = dram.tile([1, E], u16, tag=f"exd_{m}", name=f"exd_{m}")
                for t in range(4):
                    nc.sync.dma_start(
                        exd[0:1, t * 2048:(t + 1) * 2048].rearrange(
                            "one (p k) -> (one p) k", p=P),
                        idx_mt[m][t][:])
                src = exd[:].bitcast(i16).rearrange(
                    "one (c q) -> (one q) c", q=16)
                for s8 in range(8):
                    nc.sync.dma_start(eidx[m][s8 * 16:(s8 + 1) * 16, :], src)

            # ---------------- stage 4: weights / SE / bias prep ----------------
            RHS = {"r": pp.tile([P, 4, 2 * C], bf16, tag="RHS_r", name="RHS_r"),
                   "q": pp.tile([P, 4, 2 * C], bf16, tag="RHS_q", name="RHS_q")}
            Wsc = {"r": pp.tile([P, 4, 2 * C], bf16, tag="Wsc_r", name="Wsc_r"),
                   "q": pp.tile([P, 4, 2 * C], bf16, tag="Wsc_q", name="Wsc_q")}
            bias = {"r": pp.tile([P, 2 * C], f32, tag="bias_r", name="bias_r"),
                    "q": pp.tile([P, 2 * C], f32, tag="bias_q", name="bias_q")}
            wse1_sb = pp.tile([P, 8, 32], f32, tag="wse1", name="wse1")
            bse1_sb = pp.tile([1, 32], f32, tag="bse1", name="bse1")
            wse2_sb = pp.tile([32, C], f32, tag="wse2", name="wse2")
            bse2_sb = pp.tile([P, 4], f32, tag="bse2", name="bse2")
            gb = {1: pp.tile([P, 1], f32, tag="gb1", name="gb1"),
                  2: pp.tile([P, 1], f32, tag="gb2", name="gb2")}
            a_r = pp.tile([P, 4], f32, tag="a_r", name="a_r")
            a_i = pp.tile([P, 4], f32, tag="a_i", name="a_i")
            nc.vector.memset(a_r[:], 1.0)
            nc.vector.memset(a_i[:], 1.0)

            with (
                tc.tile_pool(name="s4", bufs=1) as s4,
                tc.tile_pool(name="ps_c", bufs=2, space="PSUM") as ps_c_p,
            ):
                wparts = {}
                for nm, src_t, lohi in (("wr1", wrgb_in, 0), ("wr2", wrgb_in, 1),
                                        ("wi1", wir_in, 0), ("wi2", wir_in, 1)):
                    wt = s4.tile([P, 4, C], f32, tag=nm, name=nm)
                    nc.sync.dma_start(
                        wt[:],
                        src_t[lohi * C:(lohi + 1) * C, :].rearrange(
                            "(k p) c -> p k c", p=P))
                    wparts[nm] = wt
                for k in range(4):
                    nc.vector.tensor_tensor(out=RHS["r"][:, k, 0:C],
                                            in0=wparts["wr1"][:, k, :],
                                            in1=wparts["wr2"][:, k, :], op=OP.add)
                    nc.vector.tensor_copy(RHS["r"][:, k, C:2 * C],
                                          wparts["wi2"][:, k, :])
                    nc.vector.tensor_copy(RHS["q"][:, k, 0:C],
                                          wparts["wr2"][:, k, :])
                    nc.vector.tensor_tensor(out=RHS["q"][:, k, C:2 * C],
                                            in0=wparts["wi1"][:, k, :],
                                            in1=wparts["wi2"][:, k, :], op=OP.add)
                # bias broadcast tiles
                brow = s4.tile([1, C], f32, tag="brow", name="brow")
                for nm, src_b, blk in (("r", brgb_in, 0), ("q", bir_in, 1)):
                    nc.sync.dma_start(brow[:], src_b[:])
                    psb2 = ps_c_p.tile([P, C], f32, space="PSUM", tag="psb2",
                                       name="psb2")
                    nc.tensor.matmul(psb2[:], ones_row[:], brow[:],
                                     start=True, stop=True)
                    nc.vector.tensor_copy(bias[nm][:, blk * C:(blk + 1) * C],
                                          psb2[:])
                    nc.vector.memset(bias[nm][:, (1 - blk) * C:(2 - blk) * C], 0.0)
                # SE weights
                nc.sync.dma_start(
                    wse1_sb[:],
                    wse1_in[:].rearrange("(k p) n -> p k n", p=P))
                nc.sync.dma_start(bse1_sb[:], bse1_in[:])
                nc.sync.dma_start(wse2_sb[:], wse2_in[:])
                nc.sync.dma_start(
                    bse2_sb[:],
                    bse2_in[:].rearrange("one (c p) -> (one p) c", p=P))
                for gi, gsrc in ((1, g1_in), (2, g2_in)):
                    grow = s4.tile([1, 1], f32, tag="grow", name="grow")
                    nc.sync.dma_start(grow[:], gsrc[:])
                    psg2 = ps_c_p.tile([P, 1], f32, space="PSUM", tag="psg2",
                                       name="psg2")
                    nc.tensor.matmul(psg2[:], ones_row[:], grow[:],
                                     start=True, stop=True)
                    nc.vector.tensor_copy(gb[gi][:], psg2[:])

            # ---------------- stage 5: GNN iterations ----------------
            SC_LIN = 0.505 / float(NT * KNN)
            SC_ABS = 0.495 / float(NT * KNN)
            xsrc = {"r": xb["r"], "q": xb["i"]}
            gates_a = {"r": a_r, "q": a_i}
            for it in range(iterations):
                # 5a: scale weights by accumulated gate products
                for tb in ("r", "q"):
                    for k in range(4):
                        nc.vector.tensor_scalar(
                            Wsc[tb][:, k, :], RHS[tb][:, k, :],
                            gates_a[tb][:, k:k + 1], None, op0=OP.mult)
                # 5b: tables -> DRAM
                tdram = {}
                for tb in ("r", "q"):
                    td = dram.tile([NT, 2 * C], bf16, tag=f"T{tb}_{it}",
                                   name=f"T{tb}_{it}")
                    tdram[tb] = td
                    with tc.tile_pool(name=f"ps_t{tb}{it}", bufs=2,
                                      space="PSUM") as ps_t_p:
                        for i in range(8):
                            tst = bigp.tile([P, 2 * C], bf16, tag="tst",
                                            name="tst", bufs=3)
                            for j in range(2):
                                pst = ps_t_p.tile([P, C], f32, space="PSUM",
                                                  tag="pst", name="pst")
                                for k in range(4):
                                    nc.tensor.matmul(
                                        pst[:],
                                        xsrc[tb][k][:, i * P:(i + 1) * P],
                                        Wsc[tb][:, k, j * C:(j + 1) * C],
                                        start=(k == 0), stop=(k == 3))
                                nc.vector.tensor_tensor(
                                    out=tst[:, j * C:(j + 1) * C], in0=pst[:],
                                    in1=bias[tb][:, j * C:(j + 1) * C], op=OP.add)
                            nc.sync.dma_start(td[i * P:(i + 1) * P, :], tst[:])
                # 5c: gathers + reduction
                with (
                    tc.tile_pool(name=f"ps_S{it}", bufs=1, space="PSUM") as ps_S_p,
                    tc.tile_pool(name=f"dabs{it}", bufs=4) as dap,
                ):
                    ps_S = {q: ps_S_p.tile([1, C], f32, space="PSUM",
                                           tag=f"S{q}", name=f"S{q}")
                            for q in ("lin_r", "abs_r", "lin_i", "abs_i")}
                    for ch in range(NCHUNK):
                        g1t = bigp.tile([P, 8, 2 * C], bf16, tag="big",
                                        name="g1t")
                        nc.gpsimd.dma_gather(
                            out_ap=g1t[:], in_ap=tdram["r"][:],
                            idxs_ap=eidx["r"][:, ch * 64:(ch + 1) * 64],
                            num_idxs=ECH, num_idxs_reg=ECH, elem_size=2 * C)
                        g2t = bigp.tile([P, 8, 2 * C], bf16, tag="big",
                                        name="g2t")
                        nc.gpsimd.dma_gather(
                            out_ap=g2t[:], in_ap=tdram["q"][:],
                            idxs_ap=eidx["i"][:, ch * 64:(ch + 1) * 64],
                            num_idxs=ECH, num_idxs_reg=ECH, elem_size=2 * C)
                        first = ch == 0
                        last = ch == NCHUNK - 1
                        for dirn, ga, gbuf, lo in (("r", g1t, g2t, 0),
                                                   ("i", g2t, g1t, C)):
                            dd = dap.tile([P, 8, C], bf16, tag="dd", name="dd")
                            nc.vector.tensor_tensor(
                                out=dd[:], in0=ga[:, :, lo:lo + C],
                                in1=gbuf[:, :, lo:lo + C], op=OP.subtract)
                            ad = dap.tile([P, 8, C], bf16, tag="dd", name="ad")
                            nc.scalar.activation(ad[:], dd[:], AF.Abs)
                            for s in range(8):
                                nc.tensor.matmul(
                                    ps_S[f"lin_{dirn}"][:], ones_bf[:],
                                    dd[:, s, :],
                                    start=(first and s == 0),
                                    stop=(last and s == 7))
                                nc.tensor.matmul(
                                    ps_S[f"abs_{dirn}"][:], ones_bf[:],
                                    ad[:, s, :],
                                    start=(first and s == 0),
                                    stop=(last and s == 7))
                    # 5d: S rows, AllReduce, chunked readback
                    arin = dram.tile([2, C], f32, tag=f"arin{it}",
                                     name=f"arin{it}")
                    arout = dram.tile([2, C], f32, tag=f"arout{it}",
                                      name=f"arout{it}")
                    for row, dirn in ((0, "r"), (1, "i")):
                        t1r = dap.tile([1, C], f32, tag="t1r", name="t1r")
                        t2r = dap.tile([1, C], f32, tag="t2r", name="t2r")
                        nc.vector.tensor_scalar(t1r[:], ps_S[f"lin_{dirn}"][:],
                                                SC_LIN, None, op0=OP.mult)
                        nc.vector.tensor_scalar(t2r[:], ps_S[f"abs_{dirn}"][:],
                                                SC_ABS, None, op0=OP.mult)
                        nc.vector.tensor_tensor(out=t1r[:], in0=t1r[:],
                                                in1=t2r[:], op=OP.add)
                        nc.sync.dma_start(arin[row:row + 1, :], t1r[:])
                    nc.gpsimd.collective_compute(
                        "AllReduce", OP.add,
                        replica_groups=[[0, 1], [2, 3], [4, 5], [6, 7]],
                        ins=[arin.opt()], outs=[arout.opt()])
                    cS = dap.tile([P, 8], f32, tag="cS", name="cS")
                    for row in range(2):
                        nc.sync.dma_start(
                            cS[:, row * 4:(row + 1) * 4],
                            arout[row:row + 1, :].rearrange(
                                "one (c p) -> (one p) c", p=P))
                    # 5e: SE MLP
                    with tc.tile_pool(name=f"ps_se{it}", bufs=1,
                                      space="PSUM") as ps_se_p:
                        ps_h1 = ps_se_p.tile([1, 32], f32, space="PSUM",
                                             tag="ps_h1", name="ps_h1")
                        for j in range(8):
                            nc.tensor.matmul(ps_h1[:], cS[:, j:j + 1],
                                             wse1_sb[:, j, :],
                                             start=(j == 0), stop=(j == 7))
                        h1r = dap.tile([1, 32], f32, tag="h1r", name="h1r")
                        nc.vector.tensor_tensor(out=h1r[:], in0=ps_h1[:],
                                                in1=bse1_sb[:], op=OP.add)
                        h1b = dap.tile([1, 32], f32, tag="h1b", name="h1b")
                        nc.vector.tensor_scalar_mul(h1b[:], h1r[:], 0.01)
                        nc.vector.tensor_tensor(out=h1r[:], in0=h1r[:],
                                                in1=h1b[:], op=OP.max)
                        h1d = dram.tile([1, 32], f32, tag=f"h1d{it}",
                                        name=f"h1d{it}")
                        nc.sync.dma_start(h1d[:], h1r[:])
                        h1T = dap.tile([32, 1], f32, tag="h1T", name="h1T")
                        nc.sync.dma_start(h1T[:],
                                          h1d[:].rearrange("a b -> b a"))
                        ps_gate = ps_se_p.tile([P, 4], f32, space="PSUM",
                                               tag="ps_gate", name="ps_gate")
                        for j in range(4):
                            nc.tensor.matmul(ps_gate[:, j:j + 1],
                                             wse2_sb[:, j * P:(j + 1) * P],
                                             h1T[:], start=True, stop=True,
                                             skip_group_check=True)
                        gpre = dap.tile([P, 4], f32, tag="gpre", name="gpre")
                        nc.vector.tensor_tensor(out=gpre[:], in0=ps_gate[:],
                                                in1=bse2_sb[:], op=OP.add)
                        gate = dap.tile([P, 4], f32, tag="gate", name="gate")
                        nc.scalar.activation(gate[:], gpre[:], AF.Sigmoid)
                        nc.vector.tensor_tensor(out=a_r[:], in0=a_r[:],
                                                in1=gate[:], op=OP.mult)
                        omg = dap.tile([P, 4], f32, tag="omg", name="omg")
                        nc.vector.tensor_scalar(omg[:], gate[:], -1.0, 1.0,
                                                op0=OP.mult, op1=OP.add)
                        nc.vector.tensor_tensor(out=a_i[:], in0=a_i[:],
                                                in1=omg[:], op=OP.mult)

            # ---------------- stage 6: output ----------------
            with tc.tile_pool(name="s6", bufs=2) as s6:
                alpha = s6.tile([P, 4], f32, tag="alpha", name="alpha")
                beta = s6.tile([P, 4], f32, tag="beta", name="beta")
                nc.vector.tensor_scalar(alpha[:], a_r[:], gb[1][:, 0:1], None,
                                        op0=OP.mult)
                nc.vector.tensor_scalar(beta[:], a_i[:], gb[2][:, 0:1], None,
                                        op0=OP.mult)
                for cc in range(4):
                    t1 = s6.tile([P, HN], f32, tag="t1", name="t1")
                    t2 = s6.tile([P, HN], f32, tag="t2", name="t2")
                    nc.vector.tensor_scalar(t1[:], phalf["r"][cc][:],
                                            alpha[:, cc:cc + 1], None,
                                            op0=OP.mult)
                    nc.vector.tensor_scalar(t2[:], phalf["i"][cc][:],
                                            beta[:, cc:cc + 1], None,
                                            op0=OP.mult)
                    nc.vector.tensor_tensor(out=t1[:], in0=t1[:], in1=t2[:],
                                            op=OP.add)
                    nc.vector.tensor_scalar_max(t1[:], t1[:], 0.0)
                    nc.sync.dma_start(out_t[cc * P:(cc + 1) * P, :], t1[:])

    nc.compile()
    return nc


def _prepare_in_maps(rgb, ir, W_rgb_g, b_rgb_g, W_ir_g, b_ir_g,
                     W_se1, b_se1, W_se2, b_se2, gamma1, gamma2):
    f32 = np.float32
    common = {
        "wrgb": np.ascontiguousarray(W_rgb_g, f32),
        "wir": np.ascontiguousarray(W_ir_g, f32),
        "brgb": np.ascontiguousarray(b_rgb_g, f32).reshape(1, C),
        "bir": np.ascontiguousarray(b_ir_g, f32).reshape(1, C),
        "wse1": np.ascontiguousarray(W_se1, f32),
        "bse1": np.ascontiguousarray(b_se1, f32).reshape(1, 32),
        "wse2": np.ascontiguousarray(W_se2, f32),
        "bse2": np.ascontiguousarray(b_se2, f32).reshape(1, C),
        "g1": np.asarray(gamma1, f32).reshape(1, 1),
        "g2": np.asarray(gamma2, f32).reshape(1, 1),
    }
    in_maps = []
    for core in range(N_CORES):
        s, hh = core // 2, core % 2
        r = np.asarray(rgb[s], f32)
        i = np.asarray(ir[s], f32)
        if hh:
            r = np.roll(r, -32, axis=1)
            i = np.roll(i, -32, axis=1)
        m = dict(common)
        m["rgb"] = np.ascontiguousarray(r)
        m["ir"] = np.ascontiguousarray(i)
        in_maps.append(m)
    return in_maps


def kernel(rgb, ir, W_rgb_g, b_rgb_g, W_ir_g, b_ir_g,
           W_se1, b_se1, W_se2, b_se2, gamma1, gamma2,
           gnn_iterations, k):
    from concourse.bass_utils import run_bass_kernel_spmd

    iterations = int(gnn_iterations)
    assert int(k) == KNN, f"kernel hardcodes k=16, got {k}"
    if iterations not in _CACHE:
        _CACHE[iterations] = _build(iterations)
    nc = _CACHE[iterations]

    in_maps = _prepare_in_maps(rgb, ir, W_rgb_g, b_rgb_g, W_ir_g, b_ir_g,
                               W_se1, b_se1, W_se2, b_se2, gamma1, gamma2)
    res = run_bass_kernel_spmd(nc, in_maps, core_ids=list(range(N_CORES)))

    out = np.empty((4, C, 32, 32), np.float32)
    for s in range(4):
        lo = res.results[2 * s]["out"].reshape(C, 16, 32)
        hi = res.results[2 * s + 1]["out"].reshape(C, 16, 32)
        out[s] = np.concatenate([lo, hi], axis=1)
    return out


# revision 3
# speedup vs baseline: 1.0877x; 1.0877x over previous
"""Trainium2 Bass kernel for nn_FCN8sAtOnceMultiGnn2 (gnn_message_passing).

Strategy (per 8 NeuronCores; sample s = core//2, node-half = core%2):
  The GNN messages only feed a per-(sample,channel) SE gate: m_r/m_i are
  consumed by a full mean over nodes, so per iteration we only need
    S[c] = sum_edges lrelu(P[r_e,c] - Q[q_e,c] + b_c)
  where P/Q are per-sample tables h @ W (h = gate-scaled pooled features).
  The final output is relu(g1*prod(gate)*rgb_pooled + g2*prod(1-gate)*ir_pooled).

  Per core: maxpool -> bf16 Gram -> top-16 via DVE max8/max_index/match_replace
  -> edge lists -> per iteration: scale weights by accumulated gates, compute
  combined tables T_r=[Wr1+Wr2 | Wi2], T_q=[Wr2 | Wi1+Wi2] applied to bf16
  features (+bias), write to DRAM, dma_gather rows at edge indices, reduce
  sum(d) and sum(|d|) per channel via ones-matmul on PE (lrelu = .505x+.495|x|),
  pairwise AllReduce partial sums, SE MLP -> gate. Host reassembles halves.
"""
import sys

sys.path.insert(0, "/opt/trn_rl_repo")

import numpy as np

_CACHE = {}

P = 128
C = 512          # channels
NT = 1024        # nodes per sample (32*32 after pool)
HN = 512         # nodes per core (half sample)
KNN = 16
E = HN * KNN     # 8192 edges per core per direction
ECH = 1024       # edges per gather chunk
NCHUNK = E // ECH
N_CORES = 8


def _build(iterations: int):
    import concourse.bacc as bacc
    import concourse.mybir as mybir
    import concourse.tile as tile

    dt = mybir.dt
    f32, bf16, i16, u16 = dt.float32, dt.bfloat16, dt.int16, dt.uint16
    AF = mybir.ActivationFunctionType
    OP = mybir.AluOpType

    nc = bacc.Bacc("TRN2", target_bir_lowering=False, debug=False,
                   num_devices=N_CORES)

    rgb_in = nc.dram_tensor("rgb", [C, 64, 64], f32, kind="ExternalInput")
    ir_in = nc.dram_tensor("ir", [C, 64, 64], f32, kind="ExternalInput")
    wrgb_in = nc.dram_tensor("wrgb", [2 * C, C], f32, kind="ExternalInput")
    wir_in = nc.dram_tensor("wir", [2 * C, C], f32, kind="ExternalInput")
    brgb_in = nc.dram_tensor("brgb", [1, C], f32, kind="ExternalInput")
    bir_in = nc.dram_tensor("bir", [1, C], f32, kind="ExternalInput")
    wse1_in = nc.dram_tensor("wse1", [2 * C, 32], f32, kind="ExternalInput")
    bse1_in = nc.dram_tensor("bse1", [1, 32], f32, kind="ExternalInput")
    wse2_in = nc.dram_tensor("wse2", [32, C], f32, kind="ExternalInput")
    bse2_in = nc.dram_tensor("bse2", [1, C], f32, kind="ExternalInput")
    g1_in = nc.dram_tensor("g1", [1, 1], f32, kind="ExternalInput")
    g2_in = nc.dram_tensor("g2", [1, 1], f32, kind="ExternalInput")
    out_t = nc.dram_tensor("out", [C, HN], f32, kind="ExternalOutput")

    MODS = ("r", "i")
    mod_in = {"r": rgb_in, "i": ir_in}

    with tile.TileContext(nc) as tc:
        with (
            tc.tile_pool(name="persist", bufs=1) as pp,
            tc.tile_pool(name="big", bufs=3) as bigp,
            tc.tile_pool(name="dram", bufs=1, space="DRAM") as dram,
        ):
            # ---------------- constants ----------------
            ones_bf = pp.tile([P, 1], bf16, tag="ones_bf")
            nc.vector.memset(ones_bf[:], 1.0)
            ones_row = pp.tile([1, P], f32, tag="ones_row")
            nc.vector.memset(ones_row[:], 1.0)

            # persistent per-(mod, chunk) tiles
            xb = {m: [pp.tile([P, NT], bf16, tag=f"xb_{m}{cc}", name=f"xb_{m}{cc}")
                      for cc in range(4)] for m in MODS}
            phalf = {m: [pp.tile([P, HN], f32, tag=f"ph_{m}{cc}", name=f"ph_{m}{cc}")
                         for cc in range(4)] for m in MODS}
            rn = {m: pp.tile([1, NT], f32, tag=f"rn_{m}", name=f"rn_{m}")
                  for m in MODS}
            rni2x = {m: pp.tile([P, 4], f32, tag=f"rni_{m}", name=f"rni_{m}")
                     for m in MODS}
            idx_mt = {m: [pp.tile([P, KNN], u16, tag=f"ix_{m}{t}", name=f"ix_{m}{t}")
                          for t in range(4)] for m in MODS}
            eidx = {m: pp.tile([P, E // 16], i16, tag=f"eix_{m}", name=f"eix_{m}")
                    for m in MODS}

            # ---------------- stage 1: load + pool + norms ----------------
            with (
                tc.tile_pool(name="s1", bufs=1) as s1,
                tc.tile_pool(name="ps_ss", bufs=1, space="PSUM") as ps_ss_p,
            ):
                ps_ss = {m: [ps_ss_p.tile([1, C], f32, space="PSUM",
                                          tag=f"ss{m}{h}", name=f"ss{m}{h}")
                             for h in range(2)] for m in MODS}
                for m in MODS:
                    for cc in range(4):
                        raw = s1.tile([P, 64, 64], f32, tag="raw", name="raw", bufs=2)
                        nc.sync.dma_start(raw[:], mod_in[m][cc * P:(cc + 1) * P])
                        h1 = s1.tile([P, 32, 64], f32, tag="h1", name="h1")
                        nc.vector.tensor_tensor(out=h1[:], in0=raw[:, 0::2, :],
                                                in1=raw[:, 1::2, :], op=OP.max)
                        pf = s1.tile([P, 32, 32], f32, tag="pf", name="pf", bufs=2)
                        nc.vector.tensor_tensor(out=pf[:], in0=h1[:, :, 0::2],
                                                in1=h1[:, :, 1::2], op=OP.max)
                        pff = pf.rearrange("p a b -> p (a b)")
                        nc.vector.tensor_copy(xb[m][cc][:], pff)
                        nc.vector.tensor_copy(phalf[m][cc][:], pff[:, 0:HN])
                        sq = s1.tile([P, NT], bf16, tag="sq", name="sq")
                        nc.vector.tensor_tensor(out=sq[:], in0=pff, in1=pff,
                                                op=OP.mult)
                        for h in range(2):
                            nc.tensor.matmul(ps_ss[m][h][:], ones_bf[:],
                                             sq[:, h * C:(h + 1) * C],
                                             start=(cc == 0), stop=(cc == 3))
                # norms: rn = 1/max(sqrt(ss), 1e-12); Drow = ss * rn^2
                for m in MODS:
                    srow = s1.tile([1, NT], f32, tag="srow", name="srow")
                    for h in range(2):
                        nc.scalar.activation(srow[:, h * C:(h + 1) * C],
                                             ps_ss[m][h][:], AF.Sqrt)
                    nc.vector.tensor_scalar_max(srow[:], srow[:], 1e-12)
                    nc.vector.reciprocal(rn[m][:], srow[:])
                    # Drow = ss * rn * rn  (reuse srow as scratch)
                    nc.vector.tensor_tensor(out=srow[:], in0=rn[m][:],
                                            in1=rn[m][:], op=OP.mult)
                    for h in range(2):
                        nc.vector.tensor_tensor(out=srow[:, h * C:(h + 1) * C],
                                                in0=srow[:, h * C:(h + 1) * C],
                                                in1=ps_ss[m][h][:], op=OP.mult)
                    drow_d = dram.tile([1, NT], f32, tag=f"drow_{m}",
                                       name=f"drow_{m}")
                    nc.sync.dma_start(drow_d[:], srow[:])
                    # rni2x: columns of rn (my nodes 0..511) * 2, via DRAM trip
                    rhd = dram.tile([1, HN], f32, tag=f"rhd_{m}", name=f"rhd_{m}")
                    nc.sync.dma_start(rhd[:], rn[m][:, 0:HN])
                    nc.sync.dma_start(
                        rni2x[m][:],
                        rhd[:].rearrange("one (c p) -> (one p) c", p=P))
                    nc.vector.tensor_scalar_mul(rni2x[m][:], rni2x[m][:], 2.0)
                    dram_drow = drow_d  # keep handle
                    rn_dram = {m: None}
                    # stash on dict for stage 2 use
                    if m == "r":
                        drow_r = drow_d
                    else:
                        drow_i = drow_d
            drow_d_map = {"r": drow_r, "i": drow_i}

            # ---------------- stage 2: Gram + nd + top16 ----------------
            with (
                tc.tile_pool(name="s2", bufs=2) as s2,
                tc.tile_pool(name="s2b", bufs=1) as s2b,
                tc.tile_pool(name="ps_g", bufs=2, space="PSUM") as ps_g_p,
                tc.tile_pool(name="ps_b", bufs=2, space="PSUM") as ps_b_p,
            ):
                B = {}
                Db = {}
                for m in MODS:
                    B[m] = s2b.tile([P, NT], f32, tag=f"B_{m}", name=f"B_{m}")
                    Db[m] = s2b.tile([P, NT], f32, tag=f"Db_{m}", name=f"Db_{m}")
                    for h in range(2):
                        psb = ps_b_p.tile([P, C], f32, space="PSUM", tag="psb",
                                          name="psb")
                        nc.tensor.matmul(psb[:], ones_row[:],
                                         rn[m][:, h * C:(h + 1) * C],
                                         start=True, stop=True)
                        nc.vector.tensor_copy(B[m][:, h * C:(h + 1) * C], psb[:])
                    # broadcast Drow from DRAM row: simple DMA with step-0 AP
                    # (read same row into all 128 partitions) is not expressible;
                    # use matmul broadcast via a staged row.
                    drow_sb = s2.tile([1, NT], f32, tag="drow_sb", name="drow_sb")
                    nc.sync.dma_start(drow_sb[:], drow_d_map[m][:])
                    for h in range(2):
                        psb = ps_b_p.tile([P, C], f32, space="PSUM", tag="psb",
                                          name="psb")
                        nc.tensor.matmul(psb[:], ones_row[:],
                                         drow_sb[:, h * C:(h + 1) * C],
                                         start=True, stop=True)
                        nc.vector.tensor_copy(Db[m][:, h * C:(h + 1) * C], psb[:])

                for m in MODS:
                    for t in range(4):
                        nd = s2.tile([P, NT], f32, tag="nd", name="nd")
                        for h in range(2):
                            psg = ps_g_p.tile([P, C], f32, space="PSUM",
                                              tag="psg", name="psg")
                            for k in range(4):
                                nc.tensor.matmul(
                                    psg[:],
                                    xb[m][k][:, t * P:(t + 1) * P],
                                    xb[m][k][:, h * C:(h + 1) * C],
                                    start=(k == 0), stop=(k == 3))
                            tmp = s2.tile([P, C], f32, tag="tmp", name="tmp")
                            nc.vector.tensor_tensor(
                                out=tmp[:], in0=psg[:],
                                in1=B[m][:, h * C:(h + 1) * C], op=OP.mult)
                            nc.vector.tensor_scalar(
                                tmp[:], tmp[:], rni2x[m][:, t:t + 1], None,
                                op0=OP.mult)
                            nc.vector.tensor_tensor(
                                out=nd[:, h * C:(h + 1) * C], in0=tmp[:],
                                in1=Db[m][:, h * C:(h + 1) * C], op=OP.subtract)
                        mx = s2.tile([P, 16], f32, tag="mx", name="mx")
                        nc.vector.max(out=mx[:, 0:8], in_=nd[:])
                        nc.vector.max_index(out=idx_mt[m][t][:, 0:8],
                                            in_max=mx[:, 0:8], in_values=nd[:])
                        nc.vector.match_replace(out=nd[:], in_to_replace=mx[:, 0:8],
                                                in_values=nd[:], imm_value=-1e30)
                        nc.vector.max(out=mx[:, 8:16], in_=nd[:])
                        nc.vector.max_index(out=idx_mt[m][t][:, 8:16],
                                            in_max=mx[:, 8:16], in_values=nd[:])

            # ---------------- stage 3: edge index lists ----------------
            for m in MODS:
                exd = dram.tile([1, E], u16, tag=f"exd_{m}", name=f"exd_{m}")
                for t in range(4):
                    nc.sync.dma_start(
                        exd[0:1, t * 2048:(t + 1) * 2048].rearrange(
                            "one (p k) -> (one p) k", p=P),
                        idx_mt[m][t][:])
                src = exd[:].bitcast(i16).rearrange(
                    "one (c q) -> (one q) c", q=16)
                for s8 in range(8):
                    nc.sync.dma_start(eidx[m][s8 * 16:(s8 + 1) * 16, :], src)

            # ---------------- stage 4: weights / SE / bias prep ----------------
            RHS = {"r": pp.tile([P, 4, 2 * C], bf16, tag="RHS_r", name="RHS_r"),
                   "q": pp.tile([P, 4, 2 * C], bf16, tag="RHS_q", name="RHS_q")}
            Wsc = {"r": pp.tile([P, 4, 2 * C], bf16, tag="Wsc_r", name="Wsc_r"),
                   "q": pp.tile([P, 4, 2 * C], bf16, tag="Wsc_q", name="Wsc_q")}
            bias = {"r": pp.tile([P, 2 * C], f32, tag="bias_r", name="bias_r"),
                    "q": pp.tile([P, 2 * C], f32, tag="bias_q", name="bias_q")}
            wse1_sb = pp.tile([P, 8, 32], f32, tag="wse1", name="wse1")
            bse1_sb = pp.tile([1, 32], f32, tag="bse1", name="bse1")
            wse2_sb = pp.tile([32, C], f32, tag="wse2", name="wse2")
            bse2_sb = pp.tile([P, 4], f32, tag="bse2", name="bse2")
            gb = {1: pp.tile([P, 1], f32, tag="gb1", name="gb1"),
                  2: pp.tile([P, 1], f32, tag="gb2", name="gb2")}
            a_r = pp.tile([P, 4], f32, tag="a_r", name="a_r")
            a_i = pp.tile([P, 4], f32, tag="a_i", name="a_i")
            nc.vector.memset(a_r[:], 1.0)
            nc.vector.memset(a_i[:], 1.0)

            with (
                tc.tile_pool(name="s4", bufs=1) as s4,
                tc.tile_pool(name="ps_c", bufs=2, space="PSUM") as ps_c_p,
            ):
                wparts = {}
                for nm, src_t, lohi in (("wr1", wrgb_in, 0), ("wr2", wrgb_in, 1),
                                        ("wi1", wir_in, 0), ("wi2", wir_in, 1)):
                    wt = s4.tile([P, 4, C], f32, tag=nm, name=nm)
                    nc.sync.dma_start(
                        wt[:],
                        src_t[lohi * C:(lohi + 1) * C, :].rearrange(
                            "(k p) c -> p k c", p=P))
                    wparts[nm] = wt
                for k in range(4):
                    nc.vector.tensor_tensor(out=RHS["r"][:, k, 0:C],
                                            in0=wparts["wr1"][:, k, :],
                                            in1=wparts["wr2"][:, k, :], op=OP.add)
                    nc.vector.tensor_copy(RHS["r"][:, k, C:2 * C],
                                          wparts["wi2"][:, k, :])
                    nc.vector.tensor_copy(RHS["q"][:, k, 0:C],
                                          wparts["wr2"][:, k, :])
                    nc.vector.tensor_tensor(out=RHS["q"][:, k, C:2 * C],
                                            in0=wparts["wi1"][:, k, :],
                                            in1=wparts["wi2"][:, k, :], op=OP.add)
                # bias broadcast tiles
                brow = s4.tile([1, C], f32, tag="brow", name="brow")
                for nm, src_b, blk in (("r", brgb_in, 0), ("q", bir_in, 1)):
                    nc.sync.dma_start(brow[:], src_b[:])
                    psb2 = ps_c_p.tile([P, C], f32, space="PSUM", tag="psb2",
                                       name="psb2")
                    nc.tensor.matmul(psb2[:], ones_row[:], brow[:],
                                     start=True, stop=True)
                    nc.vector.tensor_copy(bias[nm][:, blk * C:(blk + 1) * C],
                                          psb2[:])
                    nc.vector.memset(bias[nm][:, (1 - blk) * C:(2 - blk) * C], 0.0)
                # SE weights
                nc.sync.dma_start(
                    wse1_sb[:],
                    wse1_in[:].rearrange("(k p) n -> p k n", p=P))
                nc.sync.dma_start(bse1_sb[:], bse1_in[:])
                nc.sync.dma_start(wse2_sb[:], wse2_in[:])
                nc.sync.dma_start(
                    bse2_sb[:],
                    bse2_in[:].rearrange("one (c p) -> (one p) c", p=P))
                for gi, gsrc in ((1, g1_in), (2, g2_in)):
                    grow = s4.tile([1, 1], f32, tag="grow", name="grow")
                    nc.sync.dma_start(grow[:], gsrc[:])
                    psg2 = ps_c_p.tile([P, 1], f32, space="PSUM", tag="psg2",
                                       name="psg2")
                    nc.tensor.matmul(psg2[:], ones_row[:], grow[:],
                                     start=True, stop=True)
                    nc.vector.tensor_copy(gb[gi][:], psg2[:])

            # ---------------- stage 5: GNN iterations ----------------
            SC_LIN = 0.505 / float(NT * KNN)
            SC_ABS = 0.495 / float(NT * KNN)
            xsrc = {"r": xb["r"], "q": xb["i"]}
            gates_a = {"r": a_r, "q": a_i}
            for it in range(iterations):
                # 5a: scale weights by accumulated gate products
                for tb in ("r", "q"):
                    for k in range(4):
                        nc.vector.tensor_scalar(
                            Wsc[tb][:, k, :], RHS[tb][:, k, :],
                            gates_a[tb][:, k:k + 1], None, op0=OP.mult)
                # 5b: tables -> DRAM
                tdram = {}
                for tb in ("r", "q"):
                    td = dram.tile([NT, 2 * C], bf16, tag=f"T{tb}_{it}",
                                   name=f"T{tb}_{it}")
                    tdram[tb] = td
                    with tc.tile_pool(name=f"ps_t{tb}{it}", bufs=2,
                                      space="PSUM") as ps_t_p:
                        for i in range(8):
                            tst = bigp.tile([P, 2 * C], bf16, tag="tst",
                                            name="tst", bufs=3)
                            for j in range(2):
                                pst = ps_t_p.tile([P, C], f32, space="PSUM",
                                                  tag="pst", name="pst")
                                for k in range(4):
                                    nc.tensor.matmul(
                                        pst[:],
                                        xsrc[tb][k][:, i * P:(i + 1) * P],
                                        Wsc[tb][:, k, j * C:(j + 1) * C],
                                        start=(k == 0), stop=(k == 3))
                                nc.vector.tensor_tensor(
                                    out=tst[:, j * C:(j + 1) * C], in0=pst[:],
                                    in1=bias[tb][:, j * C:(j + 1) * C], op=OP.add)
                            nc.sync.dma_start(td[i * P:(i + 1) * P, :], tst[:])
                # 5c: gathers + reduction
                with (
                    tc.tile_pool(name=f"ps_S{it}", bufs=1, space="PSUM") as ps_S_p,
                    tc.tile_pool(name=f"dabs{it}", bufs=4) as dap,
                ):
                    ps_S = {q: ps_S_p.tile([1, C], f32, space="PSUM",
                                           tag=f"S{q}", name=f"S{q}")
                            for q in ("lin_r", "abs_r", "lin_i", "abs_i")}
                    for ch in range(NCHUNK):
                        g1t = bigp.tile([P, 8, 2 * C], bf16, tag="big",
                                        name="g1t")
                        nc.gpsimd.dma_gather(
                            out_ap=g1t[:], in_ap=tdram["r"][:],
                            idxs_ap=eidx["r"][:, ch * 64:(ch + 1) * 64],
                            num_idxs=ECH, num_idxs_reg=ECH, elem_size=2 * C)
                        g2t = bigp.tile([P, 8, 2 * C], bf16, tag="big",
                                        name="g2t")
                        nc.gpsimd.dma_gather(
                            out_ap=g2t[:], in_ap=tdram["q"][:],
                            idxs_ap=eidx["i"][:, ch * 64:(ch + 1) * 64],
                            num_idxs=ECH, num_idxs_reg=ECH, elem_size=2 * C)
                        first = ch == 0
                        last = ch == NCHUNK - 1
                        for dirn, ga, gbuf, lo in (("r", g1t, g2t, 0),
                                                   ("i", g2t, g1t, C)):
                            dd = dap.tile([P, 8, C], bf16, tag="dd", name="dd")
                            nc.vector.tensor_tensor(
                                out=dd[:], in0=ga[:, :, lo:lo + C],
                                in1=gbuf[:, :, lo:lo + C], op=OP.subtract)
                            ad = dap.tile([P, 8, C], bf16, tag="dd", name="ad")
                            nc.scalar.activation(ad[:], dd[:], AF.Abs)
                            for s in range(8):
                                nc.tensor.matmul(
                                    ps_S[f"lin_{dirn}"][:], ones_bf[:],
                                    dd[:, s, :],
                                    start=(first and s == 0),
                                    stop=(last and s == 7))
                                nc.tensor.matmul(
                                    ps_S[f"abs_{dirn}"][:], ones_bf[:],
                                    ad[:, s, :],
                                    start=(first and s == 0),
                                    stop=(last and s == 7))
                    # 5d: S rows, AllReduce, chunked readback
                    arin = dram.tile([2, C], f32, tag=f"arin{it}",
                                     name=f"arin{it}")
                    arout = dram.tile([2, C], f32, tag=f"arout{it}",
                                      name=f"arout{it}")
                    for row, dirn in ((0, "r"), (1, "i")):
                        t1r = dap.tile([1, C], f32, tag="t1r", name="t1r")
                        t2r = dap.tile([1, C], f32, tag="t2r", name="t2r")
                        nc.vector.tensor_scalar(t1r[:], ps_S[f"lin_{dirn}"][:],
                                                SC_LIN, None, op0=OP.mult)
                        nc.vector.tensor_scalar(t2r[:], ps_S[f"abs_{dirn}"][:],
                                                SC_ABS, None, op0=OP.mult)
                        nc.vector.tensor_tensor(out=t1r[:], in0=t1r[:],
                                                in1=t2r[:], op=OP.add)
                        nc.sync.dma_start(arin[row:row + 1, :], t1r[:])
                    nc.gpsimd.collective_compute(
                        "AllReduce", OP.add,
                        replica_groups=[[0, 1], [2, 3], [4, 5], [6, 7]],
                        ins=[arin.opt()], outs=[arout.opt()])
                    cS = dap.tile([P, 8], f32, tag="cS", name="cS")
                    for row in range(2):
                        nc.sync.dma_start(
                            cS[:, row * 4:(row + 1) * 4],
                            arout[row:row + 1, :].rearrange(
                                "one (c p) -> (one p) c", p=P))
                    # 5e: SE MLP
                    with tc.tile_pool(name=f"ps_se{it}", bufs=1,
                                      space="PSUM") as ps_se_p:
                        ps_h1 = ps_se_p.tile([1, 32], f32, space="PSUM",
                                             tag="ps_h1", name="ps_h1")
                        for j in range(8):
                            nc.tensor.matmul(ps_h1[:], cS[:, j:j + 1],
                                             wse1_sb[:, j, :],
                                             start=(j == 0), stop=(j == 7))
                        h1r = dap.tile([1, 32], f32, tag="h1r", name="h1r")
                        nc.vector.tensor_tensor(out=h1r[:], in0=ps_h1[:],
                                                in1=bse1_sb[:], op=OP.add)
                        h1b = dap.tile([1, 32], f32, tag="h1b", name="h1b")
                        nc.vector.tensor_scalar_mul(h1b[:], h1r[:], 0.01)
                        nc.vector.tensor_tensor(out=h1r[:], in0=h1r[:],
                                                in1=h1b[:], op=OP.max)
                        h1d = dram.tile([1, 32], f32, tag=f"h1d{it}",
                                        name=f"h1d{it}")
                        nc.sync.dma_start(h1d[:], h1r[:])
                        h1T = dap.tile([32, 1], f32, tag="h1T", name="h1T")
                        nc.sync.dma_start(h1T[:],
                                          h1d[:].rearrange("a b -> b a"))
                        ps_gate = ps_se_p.tile([P, 4], f32, space="PSUM",
                                               tag="ps_gate", name="ps_gate")
                        for j in range(4):
                            nc.tensor.matmul(ps_gate[:, j:j + 1],
                                             wse2_sb[:, j * P:(j + 1) * P],
                                             h1T[:], start=True, stop=True,
                                             skip_group_check=True)
                        gpre = dap.tile([P, 4], f32, tag="gpre", name="gpre")
                        nc.vector.tensor_tensor(out=gpre[:], in0=ps_gate[:],
                                                in1=bse2_sb[:], op=OP.add)
                        gate = dap.tile([P, 4], f32, tag="gate", name="gate")
                        nc.scalar.activation(gate[:], gpre[:], AF.Sigmoid)
                        nc.vector.tensor_tensor(out=a_r[:], in0=a_r[:],
                                                in1=gate[:], op=OP.mult)
                        omg = dap.tile([P, 4], f32, tag="omg", name="omg")
                        nc.vector.tensor_scalar(omg[:], gate[:], -1.0, 1.0,
                                                op0=OP.mult, op1=OP.add)
                        nc.vector.tensor_tensor(out=a_i[:], in0=a_i[:],
                                                in1=omg[:], op=OP.mult)

            # ---------------- stage 6: output ----------------
            with tc.tile_pool(name="s6", bufs=2) as s6:
                alpha = s6.tile([P, 4], f32, tag="alpha", name="alpha")
                beta = s6.tile([P, 4], f32, tag="beta", name="beta")
                nc.vector.tensor_scalar(alpha[:], a_r[:], gb[1][:, 0:1], None,
                                        op0=OP.mult)
                nc.vector.tensor_scalar(beta[:], a_i[:], gb[2][:, 0:1], None,
                                        op0=OP.mult)
                for cc in range(4):
                    t1 = s6.tile([P, HN], f32, tag="t1", name="t1")
                    t2 = s6.tile([P, HN], f32, tag="t2", name="t2")
                    nc.vector.tensor_scalar(t1[:], phalf["r"][cc][:],
                                            alpha[:, cc:cc + 1], None,
                                            op0=OP.mult)
                    nc.vector.tensor_scalar(t2[:], phalf["i"][cc][:],
                                            beta[:, cc:cc + 1], None,
                                            op0=OP.mult)
                    nc.vector.tensor_tensor(out=t1[:], in0=t1[:], in1=t2[:],
                                            op=OP.add)
                    nc.vector.tensor_scalar_max(t1[:], t1[:], 0.0)
                    nc.sync.dma_start(out_t[cc * P:(cc + 1) * P, :], t1[:])

    nc.compile()
    return nc


def _prepare_in_maps(rgb, ir, W_rgb_g, b_rgb_g, W_ir_g, b_ir_g,
                     W_se1, b_se1, W_se2, b_se2, gamma1, gamma2):
    f32 = np.float32
    common = {
        "wrgb": np.ascontiguousarray(W_rgb_g, f32),
        "wir": np.ascontiguousarray(W_ir_g, f32),
        "brgb": np.ascontiguousarray(b_rgb_g, f32).reshape(1, C),
        "bir": np.ascontiguousarray(b_ir_g, f32).reshape(1, C),
        "wse1": np.ascontiguousarray(W_se1, f32),
        "bse1": np.ascontiguousarray(b_se1, f32).reshape(1, 32),
        "wse2": np.ascontiguousarray(W_se2, f32),
        "bse2": np.ascontiguousarray(b_se2, f32).reshape(1, C),
        "g1": np.asarray(gamma1, f32).reshape(1, 1),
        "g2": np.asarray(gamma2, f32).reshape(1, 1),
    }
    in_maps = []
    for core in range(N_CORES):
        s, hh = core // 2, core % 2
        r = np.asarray(rgb[s], f32)
        i = np.asarray(ir[s], f32)
        if hh:
            r = np.roll(r, -32, axis=1)
            i = np.roll(i, -32, axis=1)
        m = dict(common)
        m["rgb"] = np.ascontiguousarray(r)
        m["ir"] = np.ascontiguousarray(i)
        in_maps.append(m)
    return in_maps


def _make_runner(nc):
    """Cached replica of bass2jax.run_bass_via_pjrt's multi-core branch so
    repeated kernel() calls skip jit retracing."""
    import jax
    import concourse.mybir as mybir
    from concourse import bass2jax as b2j
    from jax.experimental.shard_map import shard_map
    from jax.sharding import Mesh, PartitionSpec

    b2j.install_neuronx_cc_hook()

    partition_name = (nc.partition_id_tensor.name
                      if nc.partition_id_tensor else None)
    in_names, out_names, out_avals, zero_outs = [], [], [], []
    for alloc in nc.m.functions[0].allocations:
        if not isinstance(alloc, mybir.MemoryLocationSet):
            continue
        name = alloc.memorylocations[0].name
        if alloc.kind == "ExternalInput":
            if name != partition_name:
                in_names.append(name)
        elif alloc.kind == "ExternalOutput":
            shape = tuple(alloc.tensor_shape)
            np_dt = mybir.dt.np(alloc.dtype)
            out_names.append(name)
            out_avals.append(jax.core.ShapedArray(shape, np_dt))
            zero_outs.append(np.zeros(shape, np_dt))

    n_params = len(in_names)
    n_outs = len(out_names)
    all_in_names = list(in_names) + list(out_names)
    if partition_name is not None:
        all_in_names.append(partition_name)
    donate = tuple(range(n_params, n_params + n_outs))

    def _body(*args):
        operands = list(args)
        if partition_name is not None:
            operands.append(b2j.partition_id_tensor())
        outs = b2j._bass_exec_p.bind(
            *operands,
            out_avals=tuple(out_avals),
            in_names=tuple(all_in_names),
            out_names=tuple(out_names),
            lowering_input_output_aliases=(),
            sim_require_finite=True,
            sim_require_nnan=True,
            nc=nc,
        )
        return tuple(outs)

    devices = jax.devices()[:N_CORES]
    mesh = Mesh(np.asarray(devices), ("core",))
    in_specs = (PartitionSpec("core"),) * (n_params + n_outs)
    out_specs = (PartitionSpec("core"),) * n_outs
    sharded = jax.jit(
        shard_map(_body, mesh=mesh, in_specs=in_specs, out_specs=out_specs,
                  check_rep=False),
        donate_argnums=donate, keep_unused=True)
    concat_zeros = [np.zeros((N_CORES * z.shape[0], *z.shape[1:]), z.dtype)
                    for z in zero_outs]

    def run(in_maps):
        concat_in = [
            np.concatenate([np.asarray(in_maps[c][nm])
                            for c in range(N_CORES)], axis=0)
            for nm in in_names
        ]
        out_arrs = sharded(*concat_in, *[z.copy() for z in concat_zeros])
        return [
            {nm: np.asarray(out_arrs[i]).reshape(
                N_CORES, *out_avals[i].shape)[c]
             for i, nm in enumerate(out_names)}
            for c in range(N_CORES)
        ]

    return run


def kernel(rgb, ir, W_rgb_g, b_rgb_g, W_ir_g, b_ir_g,
           W_se1, b_se1, W_se2, b_se2, gamma1, gamma2,
           gnn_iterations, k):
    iterations = int(gnn_iterations)
    assert int(k) == KNN, f"kernel hardcodes k=16, got {k}"
    if iterations not in _CACHE:
        nc = _build(iterations)
        _CACHE[iterations] = _make_runner(nc)
    run = _CACHE[iterations]

    in_maps = _prepare_in_maps(rgb, ir, W_rgb_g, b_rgb_g, W_ir_g, b_ir_g,
                               W_se1, b_se1, W_se2, b_se2, gamma1, gamma2)
    results = run(in_maps)

    out = np.empty((4, C, 32, 32), np.float32)
    for s in range(4):
        lo = results[2 * s]["out"].reshape(C, 16, 32)
        hi = results[2 * s + 1]["out"].reshape(C, 16, 32)
        out[s] = np.concatenate([lo, hi], axis=1)
    return out


# revision 26
# speedup vs baseline: 5770.2330x; 5304.9091x over previous
"""Trainium2 Bass kernel for nn_FCN8sAtOnceMultiGnn2 (gnn_message_passing).

Strategy (8 NeuronCores; sample s = core//2, node-half = core%2):
  The GNN messages only feed a per-(sample,channel) SE gate: m_r/m_i are
  consumed by a full mean over nodes, so per iteration we only need
    S[c] = sum_edges lrelu(P[r_e,c] - Q[q_e,c] + b_c)
  where P/Q are per-sample tables h @ W (h = gate-scaled pooled features).
  The final output is relu(g1*prod(gate)*rgb_pooled + g2*prod(1-gate)*ir_pooled).

  Per core: maxpool -> bf16 Gram -> top-16 via DVE max8/max_index/match_replace
  -> edge lists -> per iteration: scale weights by accumulated gate products,
  compute combined tables T_r=[Wr1+Wr2 | Wi2], T_q=[Wr2 | Wi1+Wi2] on the PE
  (+bias), cast fp8, write to DRAM, dma_gather rows at the 8192 edge indices,
  d = sub (DVE), |d| = Abs (ACT), reduce per channel with ones-matmuls on PE
  accumulating in PSUM (lrelu sum = .505*sum(d)+.495*sum|d|), pairwise
  AllReduce the [2,512] partial sums, SE MLP -> gate. Host reassembles halves.
"""
import sys

sys.path.insert(0, "/opt/trn_rl_repo")

import numpy as np

_CACHE = {}

P = 128
C = 512          # channels
NT = 1024        # nodes per sample (32*32 after pool)
HN = 512         # nodes per core (half sample)
KNN = 16
E = HN * KNN     # 8192 edges per core per direction
ECH = 1024       # edges per gather chunk
NCHUNK = E // ECH
N_CORES = 8


def _build(iterations: int, timing: bool = False, gdt: str = 'f8'):
    from contextlib import ExitStack

    import concourse.bacc as bacc
    import concourse.bass as bass
    import concourse.mybir as mybir
    import concourse.tile as tile

    dt = mybir.dt
    f32, bf16, i16, u16, f8 = (dt.float32, dt.bfloat16, dt.int16, dt.uint16,
                               dt.float8e4)
    if gdt == 'bf16':
        f8 = bf16
    AF = mybir.ActivationFunctionType
    OP = mybir.AluOpType

    nc = bacc.Bacc("TRN2", target_bir_lowering=False, debug=False,
                   num_devices=1 if timing else N_CORES)

    rgb_in = nc.dram_tensor("rgb", [C, 64, 64], f32, kind="ExternalInput")
    ir_in = nc.dram_tensor("ir", [C, 64, 64], f32, kind="ExternalInput")
    wrgb_in = nc.dram_tensor("wrgb", [2 * C, C], f32, kind="ExternalInput")
    wir_in = nc.dram_tensor("wir", [2 * C, C], f32, kind="ExternalInput")
    brgb_in = nc.dram_tensor("brgb", [1, C], f32, kind="ExternalInput")
    bir_in = nc.dram_tensor("bir", [1, C], f32, kind="ExternalInput")
    wse1_in = nc.dram_tensor("wse1", [2 * C, 32], f32, kind="ExternalInput")
    bse1_in = nc.dram_tensor("bse1", [1, 32], f32, kind="ExternalInput")
    wse2_in = nc.dram_tensor("wse2", [32, C], f32, kind="ExternalInput")
    bse2_in = nc.dram_tensor("bse2", [1, C], f32, kind="ExternalInput")
    g1_in = nc.dram_tensor("g1", [1, 1], f32, kind="ExternalInput")
    g2_in = nc.dram_tensor("g2", [1, 1], f32, kind="ExternalInput")
    out_t = nc.dram_tensor("out", [C, HN], f32, kind="ExternalOutput")

    MODS = ("r", "i")
    mod_in = {"r": rgb_in, "i": ir_in}

    with tile.TileContext(nc) as tc:
        with (
            tc.tile_pool(name="persist", bufs=1) as pp,
            tc.tile_pool(name="big", bufs=3) as bigp,
            tc.tile_pool(name="dram", bufs=1, space="DRAM") as dram,
        ):
            # ---------------- constants / persistent tiles ----------------
            ones_bf = pp.tile([P, 1], bf16, tag="ones_bf")
            nc.vector.memset(ones_bf[:], 1.0)
            ones_row = pp.tile([1, P], f32, tag="ones_row")
            nc.vector.memset(ones_row[:], 1.0)

            xb = {m: [pp.tile([P, NT], bf16, tag=f"xb_{m}{cc}", name=f"xb_{m}{cc}")
                      for cc in range(4)] for m in MODS}
            phalf = {m: [pp.tile([P, HN], f32, tag=f"ph_{m}{cc}", name=f"ph_{m}{cc}")
                         for cc in range(4)] for m in MODS}
            rn = {m: pp.tile([1, NT], f32, tag=f"rn_{m}", name=f"rn_{m}")
                  for m in MODS}
            rni2x = {m: pp.tile([P, 4], f32, tag=f"rni_{m}", name=f"rni_{m}")
                     for m in MODS}
            idx_mt = {m: [pp.tile([P, KNN], u16, tag=f"ix_{m}{t}", name=f"ix_{m}{t}")
                          for t in range(4)] for m in MODS}
            eidx3 = pp.tile([P, 8, 128], i16, tag="eix", name="eix")
            Wc = {"r": pp.tile([P, 4, 2 * C], bf16, tag="Wc_r", name="Wc_r"),
                  "q": pp.tile([P, 4, 2 * C], bf16, tag="Wc_q", name="Wc_q")}
            bias = {"r": pp.tile([P, 2 * C], f32, tag="bias_r", name="bias_r"),
                    "q": pp.tile([P, 2 * C], f32, tag="bias_q", name="bias_q")}
            wse1_sb = pp.tile([P, 8, 32], f32, tag="wse1", name="wse1")
            bse1_sb = pp.tile([32, 1], f32, tag="bse1", name="bse1")
            wse2_sb = pp.tile([32, C], f32, tag="wse2", name="wse2")
            bse2_sb = pp.tile([P, 4], f32, tag="bse2", name="bse2")
            gb = {1: pp.tile([P, 1], f32, tag="gb1", name="gb1"),
                  2: pp.tile([P, 1], f32, tag="gb2", name="gb2")}
            a_r = pp.tile([P, 4], f32, tag="a_r", name="a_r")
            a_i = pp.tile([P, 4], f32, tag="a_i", name="a_i")
            nc.vector.memset(a_r[:], 1.0)
            nc.vector.memset(a_i[:], 1.0)

            # ---------------- weights / SE / bias prep ----------------
            with (
                tc.tile_pool(name="s4", bufs=1) as s4,
                tc.tile_pool(name="ps_c", bufs=2, space="PSUM") as ps_c_p,
            ):
                wparts = {}
                for nm, src_t, lohi in (("wr1", wrgb_in, 0), ("wr2", wrgb_in, 1),
                                        ("wi1", wir_in, 0), ("wi2", wir_in, 1)):
                    wt = s4.tile([P, 4, C], f32, tag=nm, name=nm)
                    nc.sync.dma_start(
                        wt[:],
                        src_t[lohi * C:(lohi + 1) * C, :].rearrange(
                            "(k p) c -> p k c", p=P))
                    wparts[nm] = wt
                for k in range(4):
                    nc.vector.tensor_tensor(out=Wc["r"][:, k, 0:C],
                                            in0=wparts["wr1"][:, k, :],
                                            in1=wparts["wr2"][:, k, :], op=OP.add)
                    nc.vector.tensor_copy(Wc["r"][:, k, C:2 * C],
                                          wparts["wi2"][:, k, :])
                    nc.vector.tensor_copy(Wc["q"][:, k, 0:C],
                                          wparts["wr2"][:, k, :])
                    nc.vector.tensor_tensor(out=Wc["q"][:, k, C:2 * C],
                                            in0=wparts["wi1"][:, k, :],
                                            in1=wparts["wi2"][:, k, :], op=OP.add)
                brow = s4.tile([1, C], f32, tag="brow", name="brow")
                for nm, src_b, blk in (("r", brgb_in, 0), ("q", bir_in, 1)):
                    nc.sync.dma_start(brow[:], src_b[:])
                    psb2 = ps_c_p.tile([P, C], f32, space="PSUM", tag="psb2",
                                       name="psb2")
                    nc.tensor.matmul(psb2[:], ones_row[:], brow[:],
                                     start=True, stop=True)
                    nc.vector.tensor_copy(bias[nm][:, blk * C:(blk + 1) * C],
                                          psb2[:])
                    nc.vector.memset(bias[nm][:, (1 - blk) * C:(2 - blk) * C], 0.0)
                nc.sync.dma_start(
                    wse1_sb[:],
                    wse1_in[:].rearrange("(k p) n -> p k n", p=P))
                nc.sync.dma_start(bse1_sb[:], bse1_in[:].rearrange("a b -> b a"))
                nc.sync.dma_start(wse2_sb[:], wse2_in[:])
                nc.sync.dma_start(
                    bse2_sb[:],
                    bse2_in[:].rearrange("one (c p) -> (one p) c", p=P))
                for gi, gsrc in ((1, g1_in), (2, g2_in)):
                    grow = s4.tile([1, 1], f32, tag="grow", name="grow")
                    nc.sync.dma_start(grow[:], gsrc[:])
                    psg2 = ps_c_p.tile([P, 1], f32, space="PSUM", tag="psg2",
                                       name="psg2")
                    nc.tensor.matmul(psg2[:], ones_row[:], grow[:],
                                     start=True, stop=True)
                    nc.vector.tensor_copy(gb[gi][:], psg2[:])

            # ---------------- stage 1 (per modality) ----------------
            drow_d_map = {}
            it0_ctx = ExitStack()
            ps_it0 = it0_ctx.enter_context(
                tc.tile_pool(name="psit0", bufs=1, space="PSUM"))
            s1_ctx = ExitStack()
            s1 = s1_ctx.enter_context(tc.tile_pool(name="s1", bufs=1))
            ps_ss_p = s1_ctx.enter_context(
                tc.tile_pool(name="ps_ss", bufs=1, space="PSUM"))

            def stage1_mod(m):
                ps_ss = [ps_ss_p.tile([1, C], f32, space="PSUM",
                                      tag=f"ss{h}", name=f"ss{m}{h}")
                         for h in range(2)]
                for cc in range(4):
                    raw = s1.tile([P, 64, 64], f32, tag="raw", name="raw",
                                  bufs=2)
                    nc.sync.dma_start(raw[:], mod_in[m][cc * P:(cc + 1) * P])
                    h1 = s1.tile([P, 32, 64], f32, tag="h1", name="h1",
                                 bufs=1)
                    nc.vector.tensor_tensor(out=h1[:], in0=raw[:, 0::2, :],
                                            in1=raw[:, 1::2, :], op=OP.max)
                    pf = s1.tile([P, 32, 32], f32, tag="pf", name="pf",
                                 bufs=2)
                    nc.vector.tensor_tensor(out=pf[:], in0=h1[:, :, 0::2],
                                            in1=h1[:, :, 1::2], op=OP.max)
                    pff = pf.rearrange("p a b -> p (a b)")
                    nc.scalar.activation(xb[m][cc][:], pff, AF.Copy)
                    nc.vector.tensor_copy(phalf[m][cc][:], pff[:, 0:HN])
                    sq = s1.tile([P, NT], bf16, tag="sq", name="sq", bufs=2)
                    nc.scalar.activation(sq[:], pff, AF.Square)
                    for h in range(2):
                        nc.tensor.matmul(ps_ss[h][:], ones_bf[:],
                                         sq[:, h * C:(h + 1) * C],
                                         start=(cc == 0), stop=(cc == 3))
                srow = s1.tile([1, NT], f32, tag="srow", name="srow")
                for h in range(2):
                    nc.scalar.activation(srow[:, h * C:(h + 1) * C],
                                         ps_ss[h][:], AF.Sqrt)
                nc.vector.tensor_scalar_max(srow[:], srow[:], 1e-12)
                nc.vector.reciprocal(rn[m][:], srow[:])
                nc.vector.tensor_tensor(out=srow[:], in0=rn[m][:],
                                        in1=rn[m][:], op=OP.mult)
                for h in range(2):
                    nc.vector.tensor_tensor(out=srow[:, h * C:(h + 1) * C],
                                            in0=srow[:, h * C:(h + 1) * C],
                                            in1=ps_ss[h][:], op=OP.mult)
                drow_d = dram.tile([1, NT], f32, tag=f"drow_{m}",
                                   name=f"drow_{m}")
                nc.sync.dma_start(drow_d[:], srow[:])
                drow_d_map[m] = drow_d
                rhd = dram.tile([1, HN], f32, tag=f"rhd_{m}", name=f"rhd_{m}")
                nc.sync.dma_start(rhd[:], rn[m][:, 0:HN])
                nc.sync.dma_start(
                    rni2x[m][:],
                    rhd[:].rearrange("one (c p) -> (one p) c", p=P))
                nc.vector.tensor_scalar_mul(rni2x[m][:], rni2x[m][:], 2.0)

            # ---------------- per-iteration phases ----------------
            SC_LIN = 0.505 / float(NT * KNN)
            SC_ABS = 0.495 / float(NT * KNN)
            xsrc = {"r": xb["r"], "q": xb["i"]}
            gates_a = {"r": a_r, "q": a_i}

            def open_iter_pool(it, ictx):
                return ictx.enter_context(
                    tc.tile_pool(name=f"psit{it}", bufs=1, space="PSUM"))

            def emit_table(it, tb, td, ps_it):
                use_w = Wc
                ro = 0 if tb == "r" else NT
                for i in range(8):
                    tst8 = bigp.tile([P, 2 * C], f8, tag="tst",
                                     name="tst8", bufs=3)
                    for j in range(2):
                        pst = ps_it.tile([P, C], f32, space="PSUM",
                                         tag="pst", name="pst", bufs=2)
                        for k in range(4):
                            nc.tensor.matmul(
                                pst[:],
                                xsrc[tb][k][:, i * P:(i + 1) * P],
                                use_w[tb][:, k, j * C:(j + 1) * C],
                                start=(k == 0), stop=(k == 3))
                        nc.vector.tensor_tensor(
                            out=tst8[:, j * C:(j + 1) * C], in0=pst[:],
                            in1=bias[tb][:, j * C:(j + 1) * C], op=OP.add)
                    nc.sync.dma_start(td[ro + i * P:ro + (i + 1) * P, :],
                                      tst8[:])

            def gather_phase(it, ictx, ps_it, tdram):
                dap = ictx.enter_context(
                    tc.tile_pool(name=f"dabs{it}", bufs=6))
                psS_p = ictx.enter_context(
                    tc.tile_pool(name=f"psS{it}", bufs=1, space="PSUM"))
                ps_S = {q: psS_p.tile([1, C], f32, space="PSUM", tag=f"S{q}",
                                      name=f"S{q}_{it}")
                        for q in ("lin_r", "abs_r", "lin_i", "abs_i")}
                for ch in range(NCHUNK):
                    g1f = bigp.tile([P, 8, 2 * C], f8, tag="big", name="g1f")
                    nc.gpsimd.dma_gather(
                        out_ap=g1f[:], in_ap=tdram[:],
                        idxs_ap=eidx3[:, ch, 0:64],
                        num_idxs=ECH, num_idxs_reg=ECH, elem_size=2 * C)
                    g2f = bigp.tile([P, 8, 2 * C], f8, tag="big", name="g2f")
                    nc.gpsimd.dma_gather(
                        out_ap=g2f[:], in_ap=tdram[:],
                        idxs_ap=eidx3[:, ch, 64:128],
                        num_idxs=ECH, num_idxs_reg=ECH, elem_size=2 * C)
                    g1t = g1f[:]
                    g2t = g2f[:]
                    first = ch == 0
                    last = ch == NCHUNK - 1
                    for dirn, ga, gbuf, lo in (("r", g1t, g2t, 0),
                                               ("i", g2t, g1t, C)):
                        dd = dap.tile([P, 8, C], bf16, tag="dd", name="dd")
                        nc.vector.tensor_tensor(
                            out=dd[:], in0=ga[:, :, lo:lo + C],
                            in1=gbuf[:, :, lo:lo + C], op=OP.subtract)
                        ad = dap.tile([P, 8, C], bf16, tag="dd", name="ad")
                        nc.scalar.activation(ad[:], dd[:], AF.Abs)
                        for s in range(8):
                            nc.tensor.matmul(
                                ps_S[f"lin_{dirn}"][:], ones_bf[:], dd[:, s, :],
                                start=(first and s == 0),
                                stop=(last and s == 7))
                            nc.tensor.matmul(
                                ps_S[f"abs_{dirn}"][:], ones_bf[:], ad[:, s, :],
                                start=(first and s == 0),
                                stop=(last and s == 7))
                # S rows, AllReduce, chunked readback
                arin = dram.tile([2, C], f32, tag=f"arin{it}", name=f"arin{it}")
                arout = dram.tile([2, C], f32, tag=f"arout{it}",
                                  name=f"arout{it}")
                for row, dirn in ((0, "r"), (1, "i")):
                    t1r = dap.tile([1, C], f32, tag="t1r", name="t1r")
                    t2r = dap.tile([1, C], f32, tag="t2r", name="t2r")
                    nc.vector.tensor_scalar(t1r[:], ps_S[f"lin_{dirn}"][:],
                                            SC_LIN, None, op0=OP.mult)
                    nc.vector.tensor_scalar(t2r[:], ps_S[f"abs_{dirn}"][:],
                                            SC_ABS, None, op0=OP.mult)
                    nc.vector.tensor_tensor(out=t1r[:], in0=t1r[:], in1=t2r[:],
                                            op=OP.add)
                    nc.sync.dma_start(arin[row:row + 1, :], t1r[:])
                if timing:
                    nc.gpsimd.dma_start(arout[:], arin[:])
                else:
                    nc.gpsimd.collective_compute(
                        "AllReduce", OP.add,
                        replica_groups=[[0, 1], [2, 3], [4, 5], [6, 7]],
                        ins=[arin.opt()], outs=[arout.opt()])
                cS = dap.tile([P, 8], f32, tag="cS", name="cS")
                for row in range(2):
                    nc.sync.dma_start(
                        cS[:, row * 4:(row + 1) * 4],
                        arout[row:row + 1, :].rearrange(
                            "one (c p) -> (one p) c", p=P))
                # SE MLP
                with tc.tile_pool(name=f"ps_se{it}", bufs=1,
                                  space="PSUM") as ps_se_p:
                    ps_h1 = ps_se_p.tile([32, 1], f32, space="PSUM",
                                         tag="ps_h1", name="ps_h1")
                    for j in range(8):
                        nc.tensor.matmul(ps_h1[:], wse1_sb[:, j, :],
                                         cS[:, j:j + 1],
                                         start=(j == 0), stop=(j == 7))
                    h1r = dap.tile([32, 1], f32, tag="h1r", name="h1r")
                    nc.vector.tensor_tensor(out=h1r[:], in0=ps_h1[:],
                                            in1=bse1_sb[:], op=OP.add)
                    h1b = dap.tile([32, 1], f32, tag="h1b", name="h1b")
                    nc.vector.tensor_scalar_mul(h1b[:], h1r[:], 0.01)
                    nc.vector.tensor_tensor(out=h1r[:], in0=h1r[:], in1=h1b[:],
                                            op=OP.max)
                    ps_gate = ps_se_p.tile([P, 4], f32, space="PSUM",
                                           tag="ps_gate", name="ps_gate")
                    for j in range(4):
                        nc.tensor.matmul(ps_gate[:, j:j + 1],
                                         wse2_sb[:, j * P:(j + 1) * P],
                                         h1r[:], start=True, stop=True,
                                         skip_group_check=True)
                    gpre = dap.tile([P, 4], f32, tag="gpre", name="gpre")
                    nc.vector.tensor_tensor(out=gpre[:], in0=ps_gate[:],
                                            in1=bse2_sb[:], op=OP.add)
                    gate = dap.tile([P, 4], f32, tag="gate", name="gate")
                    nc.scalar.activation(gate[:], gpre[:], AF.Sigmoid)
                    nc.vector.tensor_tensor(out=a_r[:], in0=a_r[:], in1=gate[:],
                                            op=OP.mult)
                    omg = dap.tile([P, 4], f32, tag="omg", name="omg")
                    nc.vector.tensor_scalar(omg[:], gate[:], -1.0, 1.0,
                                            op0=OP.mult, op1=OP.add)
                    nc.vector.tensor_tensor(out=a_i[:], in0=a_i[:], in1=omg[:],
                                            op=OP.mult)
                    # fold the new gate into the combined weights in place:
                    # W_t+1 = gate_t (x) W_t along the contraction channels
                    for tb, gv in (("r", gate), ("q", omg)):
                        for k in range(4):
                            nc.vector.tensor_scalar(
                                Wc[tb][:, k, :], Wc[tb][:, k, :],
                                gv[:, k:k + 1], None, op0=OP.mult)

            # main flow: per-modality pipeline; iteration-0 tables are
            # emitted right after the modality they depend on loads, so
            # PE/DMA table work overlaps the Gram/top-k phase.
            tdram0 = dram.tile([2 * NT, 2 * C], f8, tag="Tc0", name="Tc0")
            exd_comb = dram.tile([1, 2 * E], u16, tag="exd", name="exd_comb")
            for m, tb in (("r", "r"), ("i", "q")):
                stage1_mod(m)
                emit_table(0, tb, tdram0, ps_it0)
            s1_ctx.close()
            with (
                tc.tile_pool(name="s2", bufs=2) as s2,
                tc.tile_pool(name="s2b", bufs=1) as s2b,
                tc.tile_pool(name="ps_g", bufs=2, space="PSUM") as ps_g_p,
            ):
                B = {}
                Db = {}
                for m in MODS:
                    B[m] = s2b.tile([P, NT], f32, tag=f"B{m}", name=f"B_{m}")
                    Db[m] = s2b.tile([P, NT], f32, tag=f"Db{m}", name=f"Db_{m}")
                    drow_sb = s2.tile([1, NT], f32, tag="drow_sb",
                                      name="drow_sb")
                    nc.sync.dma_start(drow_sb[:], drow_d_map[m][:])
                    for h in range(2):
                        psb = ps_g_p.tile([P, C], f32, space="PSUM", tag="psg",
                                          name="psb")
                        nc.tensor.matmul(psb[:], ones_row[:],
                                         rn[m][:, h * C:(h + 1) * C],
                                         start=True, stop=True)
                        nc.vector.tensor_copy(B[m][:, h * C:(h + 1) * C],
                                              psb[:])
                        psb = ps_g_p.tile([P, C], f32, space="PSUM", tag="psg",
                                          name="psb2")
                        nc.tensor.matmul(psb[:], ones_row[:],
                                         drow_sb[:, h * C:(h + 1) * C],
                                         start=True, stop=True)
                        nc.vector.tensor_copy(Db[m][:, h * C:(h + 1) * C],
                                              psb[:])
                for t in range(4):
                    for m in MODS:
                        moff = 0 if m == "r" else 1024
                        nd = s2.tile([P, NT], f32, tag="nd", name="nd")
                        for h in range(2):
                            psg = ps_g_p.tile([P, C], f32, space="PSUM",
                                              tag="psg", name="psg")
                            for k in range(4):
                                nc.tensor.matmul(
                                    psg[:],
                                    xb[m][k][:, t * P:(t + 1) * P],
                                    xb[m][k][:, h * C:(h + 1) * C],
                                    start=(k == 0), stop=(k == 3))
                            tmp = s2.tile([P, C], f32, tag="tmp", name="tmp")
                            nc.vector.tensor_tensor(
                                out=tmp[:], in0=psg[:],
                                in1=B[m][:, h * C:(h + 1) * C], op=OP.mult)
                            nc.vector.scalar_tensor_tensor(
                                out=nd[:, h * C:(h + 1) * C], in0=tmp[:],
                                scalar=rni2x[m][:, t:t + 1],
                                in1=Db[m][:, h * C:(h + 1) * C],
                                op0=OP.mult, op1=OP.subtract)
                        mx = s2.tile([P, 16], f32, tag="mx", name="mx")
                        nc.vector.max(out=mx[:, 0:8], in_=nd[:])
                        nc.vector.max_index(out=idx_mt[m][t][:, 0:8],
                                            in_max=mx[:, 0:8], in_values=nd[:])
                        nc.vector.match_replace(out=nd[:],
                                                in_to_replace=mx[:, 0:8],
                                                in_values=nd[:],
                                                imm_value=-1e30)
                        nc.vector.max(out=mx[:, 8:16], in_=nd[:])
                        nc.vector.max_index(out=idx_mt[m][t][:, 8:16],
                                            in_max=mx[:, 8:16], in_values=nd[:])
                        # stage the tile's edge list; modality i shifted +NT
                        # (second half of the combined gather table)
                        if m == "i":
                            sh = s2.tile([P, KNN], u16, tag="sh", name="sh")
                            nc.vector.tensor_scalar(
                                sh[:], idx_mt[m][t][:], NT, None, op0=OP.add)
                            wsrc = sh
                        else:
                            wsrc = idx_mt[m][t]
                        for hf in range(2):
                            chn = 2 * t + hf
                            base = chn * 2048 + moff
                            dst = exd_comb[0:1, base:base + 1024].rearrange(
                                "one (p k) -> (one p) k", p=64)
                            nc.sync.dma_start(
                                dst, wsrc[hf * 64:(hf + 1) * 64, :])
                    # replicated idx stripes for chunks 2t, 2t+1
                    srcidx = exd_comb[0:1, t * 4096:(t + 1) * 4096].bitcast(
                        i16).rearrange("one (c q) -> (one q) c", q=16)
                    for s8 in range(8):
                        nc.sync.dma_start(
                            eidx3[s8 * 16:(s8 + 1) * 16, 2 * t:2 * t + 2, :],
                            srcidx)

            gather_phase(0, it0_ctx, ps_it0, tdram0)
            it0_ctx.close()
            for it in range(1, iterations):
                ictx = ExitStack()
                ps_it = open_iter_pool(it, ictx)
                tdram = dram.tile([2 * NT, 2 * C], f8, tag=f"Tc{it}",
                                  name=f"Tc{it}")
                for tb in ("r", "q"):
                    emit_table(it, tb, tdram, ps_it)
                gather_phase(it, ictx, ps_it, tdram)
                ictx.close()

            # ---------------- output ----------------
            with tc.tile_pool(name="s6", bufs=2) as s6:
                alpha = s6.tile([P, 4], f32, tag="alpha", name="alpha")
                beta = s6.tile([P, 4], f32, tag="beta", name="beta")
                nc.vector.tensor_scalar(alpha[:], a_r[:], gb[1][:, 0:1], None,
                                        op0=OP.mult)
                nc.vector.tensor_scalar(beta[:], a_i[:], gb[2][:, 0:1], None,
                                        op0=OP.mult)
                for cc in range(4):
                    t1 = s6.tile([P, HN], f32, tag="t1", name="t1")
                    t2 = s6.tile([P, HN], f32, tag="t2", name="t2")
                    nc.vector.tensor_scalar(t1[:], phalf["r"][cc][:],
                                            alpha[:, cc:cc + 1], None,
                                            op0=OP.mult)
                    nc.vector.tensor_scalar(t2[:], phalf["i"][cc][:],
                                            beta[:, cc:cc + 1], None,
                                            op0=OP.mult)
                    nc.vector.tensor_tensor(out=t1[:], in0=t1[:], in1=t2[:],
                                            op=OP.add)
                    nc.vector.tensor_scalar_max(t1[:], t1[:], 0.0)
                    nc.sync.dma_start(out_t[cc * P:(cc + 1) * P, :], t1[:])

    nc.compile()
    return nc


def _prepare_in_maps(rgb, ir, W_rgb_g, b_rgb_g, W_ir_g, b_ir_g,
                     W_se1, b_se1, W_se2, b_se2, gamma1, gamma2):
    f32 = np.float32
    common = {
        "wrgb": np.ascontiguousarray(W_rgb_g, f32),
        "wir": np.ascontiguousarray(W_ir_g, f32),
        "brgb": np.ascontiguousarray(b_rgb_g, f32).reshape(1, C),
        "bir": np.ascontiguousarray(b_ir_g, f32).reshape(1, C),
        "wse1": np.ascontiguousarray(W_se1, f32),
        "bse1": np.ascontiguousarray(b_se1, f32).reshape(1, 32),
        "wse2": np.ascontiguousarray(W_se2, f32),
        "bse2": np.ascontiguousarray(b_se2, f32).reshape(1, C),
        "g1": np.asarray(gamma1, f32).reshape(1, 1),
        "g2": np.asarray(gamma2, f32).reshape(1, 1),
    }
    in_maps = []
    for core in range(N_CORES):
        s, hh = core // 2, core % 2
        r = np.asarray(rgb[s], f32)
        i = np.asarray(ir[s], f32)
        if hh:
            r = np.roll(r, -32, axis=1)
            i = np.roll(i, -32, axis=1)
        m = dict(common)
        m["rgb"] = np.ascontiguousarray(r)
        m["ir"] = np.ascontiguousarray(i)
        in_maps.append(m)
    return in_maps


def _make_runner(nc):
    """Cached replica of bass2jax.run_bass_via_pjrt's multi-core branch so
    repeated kernel() calls skip jit retracing."""
    import jax
    import concourse.mybir as mybir
    from concourse import bass2jax as b2j
    from jax.experimental.shard_map import shard_map
    from jax.sharding import Mesh, PartitionSpec

    b2j.install_neuronx_cc_hook()

    partition_name = (nc.partition_id_tensor.name
                      if nc.partition_id_tensor else None)
    in_names, out_names, out_avals, zero_outs = [], [], [], []
    for alloc in nc.m.functions[0].allocations:
        if not isinstance(alloc, mybir.MemoryLocationSet):
            continue
        name = alloc.memorylocations[0].name
        if alloc.kind == "ExternalInput":
            if name != partition_name:
                in_names.append(name)
        elif alloc.kind == "ExternalOutput":
            shape = tuple(alloc.tensor_shape)
            np_dt = mybir.dt.np(alloc.dtype)
            out_names.append(name)
            out_avals.append(jax.core.ShapedArray(shape, np_dt))
            zero_outs.append(np.zeros(shape, np_dt))

    n_params = len(in_names)
    n_outs = len(out_names)
    all_in_names = list(in_names) + list(out_names)
    if partition_name is not None:
        all_in_names.append(partition_name)
    donate = tuple(range(n_params, n_params + n_outs))

    def _body(*args):
        operands = list(args)
        if partition_name is not None:
            operands.append(b2j.partition_id_tensor())
        outs = b2j._bass_exec_p.bind(
            *operands,
            out_avals=tuple(out_avals),
            in_names=tuple(all_in_names),
            out_names=tuple(out_names),
            lowering_input_output_aliases=(),
            sim_require_finite=True,
            sim_require_nnan=True,
            nc=nc,
        )
        return tuple(outs)

    devices = jax.devices()[:N_CORES]
    mesh = Mesh(np.asarray(devices), ("core",))
    in_specs = (PartitionSpec("core"),) * (n_params + n_outs)
    out_specs = (PartitionSpec("core"),) * n_outs
    sharded = jax.jit(
        shard_map(_body, mesh=mesh, in_specs=in_specs, out_specs=out_specs,
                  check_rep=False),
        donate_argnums=donate, keep_unused=True)
    concat_zeros = [np.zeros((N_CORES * z.shape[0], *z.shape[1:]), z.dtype)
                    for z in zero_outs]

    def run(in_maps):
        concat_in = [
            np.concatenate([np.asarray(in_maps[c][nm])
                            for c in range(N_CORES)], axis=0)
            for nm in in_names
        ]
        out_arrs = sharded(*concat_in, *[z.copy() for z in concat_zeros])
        return [
            {nm: np.asarray(out_arrs[i]).reshape(
                N_CORES, *out_avals[i].shape)[c]
             for i, nm in enumerate(out_names)}
            for c in range(N_CORES)
        ]

    return run


def kernel(rgb, ir, W_rgb_g, b_rgb_g, W_ir_g, b_ir_g,
           W_se1, b_se1, W_se2, b_se2, gamma1, gamma2,
           gnn_iterations, k):
    iterations = int(gnn_iterations)
    assert int(k) == KNN, f"kernel hardcodes k=16, got {k}"
    if iterations not in _CACHE:
        import os
        nc = _build(iterations, gdt=os.environ.get("GATHER_DTYPE", "f8"))
        _CACHE[iterations] = _make_runner(nc)
    run = _CACHE[iterations]

    in_maps = _prepare_in_maps(rgb, ir, W_rgb_g, b_rgb_g, W_ir_g, b_ir_g,
                               W_se1, b_se1, W_se2, b_se2, gamma1, gamma2)
    results = run(in_maps)

    out = np.empty((4, C, 32, 32), np.float32)
    for s in range(4):
        lo = results[2 * s]["out"].reshape(C, 16, 32)
        hi = results[2 * s + 1]["out"].reshape(C, 16, 32)
        out[s] = np.concatenate([lo, hi], axis=1)
    return out


# revision 28
# speedup vs baseline: 7058.2508x; 1.2232x over previous
"""Trainium2 Bass kernel for nn_FCN8sAtOnceMultiGnn2 (gnn_message_passing).

Strategy (8 NeuronCores; sample s = core//2, node-half = core%2):
  The GNN messages only feed a per-(sample,channel) SE gate: m_r/m_i are
  consumed by a full mean over nodes, so per iteration we only need
    S[c] = sum_edges lrelu(P[r_e,c] - Q[q_e,c] + b_c)
  where P/Q are per-sample tables h @ W (h = gate-scaled pooled features).
  The final output is relu(g1*prod(gate)*rgb_pooled + g2*prod(1-gate)*ir_pooled).

  Per core: maxpool -> bf16 Gram -> top-16 via DVE max8/max_index/match_replace
  -> edge lists -> per iteration: scale weights by accumulated gate products,
  compute combined tables T_r=[Wr1+Wr2 | Wi2], T_q=[Wr2 | Wi1+Wi2] on the PE
  (+bias), cast fp8, write to DRAM, dma_gather rows at the 8192 edge indices,
  d = sub (DVE), |d| = Abs (ACT), reduce per channel with ones-matmuls on PE
  accumulating in PSUM (lrelu sum = .505*sum(d)+.495*sum|d|), pairwise
  AllReduce the [2,512] partial sums, SE MLP -> gate. Host reassembles halves.
"""
import sys

sys.path.insert(0, "/opt/trn_rl_repo")

import numpy as np

_CACHE = {}

P = 128
C = 512          # channels
NT = 1024        # nodes per sample (32*32 after pool)
HN = 512         # nodes per core (half sample)
KNN = 16
E = HN * KNN     # 8192 edges per core per direction
ECH = 1024       # edges per gather chunk
NCHUNK = E // ECH
N_CORES = 8


def _build(iterations: int, timing: bool = False, gdt: str = 'f8'):
    from contextlib import ExitStack

    import concourse.bacc as bacc
    import concourse.bass as bass
    import concourse.mybir as mybir
    import concourse.tile as tile

    dt = mybir.dt
    f32, bf16, i16, u16, f8 = (dt.float32, dt.bfloat16, dt.int16, dt.uint16,
                               dt.float8e4)
    if gdt == 'bf16':
        f8 = bf16
    AF = mybir.ActivationFunctionType
    OP = mybir.AluOpType

    nc = bacc.Bacc("TRN2", target_bir_lowering=False, debug=False,
                   num_devices=1 if timing else N_CORES)

    rgb_in = nc.dram_tensor("rgb", [C, 64, 64], f32, kind="ExternalInput")
    ir_in = nc.dram_tensor("ir", [C, 64, 64], f32, kind="ExternalInput")
    wrgb_in = nc.dram_tensor("wrgb", [2 * C, C], f32, kind="ExternalInput")
    wir_in = nc.dram_tensor("wir", [2 * C, C], f32, kind="ExternalInput")
    brgb_in = nc.dram_tensor("brgb", [1, C], f32, kind="ExternalInput")
    bir_in = nc.dram_tensor("bir", [1, C], f32, kind="ExternalInput")
    wse1_in = nc.dram_tensor("wse1", [2 * C, 32], f32, kind="ExternalInput")
    bse1_in = nc.dram_tensor("bse1", [1, 32], f32, kind="ExternalInput")
    wse2_in = nc.dram_tensor("wse2", [32, C], f32, kind="ExternalInput")
    bse2_in = nc.dram_tensor("bse2", [1, C], f32, kind="ExternalInput")
    g1_in = nc.dram_tensor("g1", [1, 1], f32, kind="ExternalInput")
    g2_in = nc.dram_tensor("g2", [1, 1], f32, kind="ExternalInput")
    out_t = nc.dram_tensor("out", [C, HN], f32, kind="ExternalOutput")

    MODS = ("r", "i")
    mod_in = {"r": rgb_in, "i": ir_in}

    with tile.TileContext(nc) as tc:
        with (
            tc.tile_pool(name="persist", bufs=1) as pp,
            tc.tile_pool(name="big", bufs=3) as bigp,
            tc.tile_pool(name="dram", bufs=1, space="DRAM") as dram,
        ):
            # ---------------- constants / persistent tiles ----------------
            ones_bf = pp.tile([P, 1], bf16, tag="ones_bf")
            nc.vector.memset(ones_bf[:], 1.0)
            ones_row = pp.tile([1, P], f32, tag="ones_row")
            nc.vector.memset(ones_row[:], 1.0)

            xb = {m: [pp.tile([P, NT], bf16, tag=f"xb_{m}{cc}", name=f"xb_{m}{cc}")
                      for cc in range(4)] for m in MODS}
            phalf = {m: [pp.tile([P, HN], f32, tag=f"ph_{m}{cc}", name=f"ph_{m}{cc}")
                         for cc in range(4)] for m in MODS}
            rn = {m: pp.tile([1, NT], f32, tag=f"rn_{m}", name=f"rn_{m}")
                  for m in MODS}
            rni2x = {m: pp.tile([P, 4], f32, tag=f"rni_{m}", name=f"rni_{m}")
                     for m in MODS}
            idx_mt = {m: [pp.tile([P, KNN], u16, tag=f"ix_{m}{t}", name=f"ix_{m}{t}")
                          for t in range(4)] for m in MODS}
            eidx3 = pp.tile([P, 8, 128], i16, tag="eix", name="eix")
            Wc = {"r": pp.tile([P, 4, 2 * C], bf16, tag="Wc_r", name="Wc_r"),
                  "q": pp.tile([P, 4, 2 * C], bf16, tag="Wc_q", name="Wc_q")}
            bias = {"r": pp.tile([P, 2 * C], f32, tag="bias_r", name="bias_r"),
                    "q": pp.tile([P, 2 * C], f32, tag="bias_q", name="bias_q")}
            wse1_sb = pp.tile([P, 8, 32], f32, tag="wse1", name="wse1")
            bse1_sb = pp.tile([32, 1], f32, tag="bse1", name="bse1")
            wse2_sb = pp.tile([32, C], f32, tag="wse2", name="wse2")
            bse2_sb = pp.tile([P, 4], f32, tag="bse2", name="bse2")
            gb = {1: pp.tile([P, 1], f32, tag="gb1", name="gb1"),
                  2: pp.tile([P, 1], f32, tag="gb2", name="gb2")}
            a_r = pp.tile([P, 4], f32, tag="a_r", name="a_r")
            a_i = pp.tile([P, 4], f32, tag="a_i", name="a_i")
            nc.vector.memset(a_r[:], 1.0)
            nc.vector.memset(a_i[:], 1.0)

            # ---------------- weights / SE / bias prep ----------------
            with (
                tc.tile_pool(name="s4", bufs=1) as s4,
                tc.tile_pool(name="ps_c", bufs=2, space="PSUM") as ps_c_p,
            ):
                wparts = {}
                for nm, src_t, lohi in (("wr1", wrgb_in, 0), ("wr2", wrgb_in, 1),
                                        ("wi1", wir_in, 0), ("wi2", wir_in, 1)):
                    wt = s4.tile([P, 4, C], f32, tag=nm, name=nm)
                    nc.sync.dma_start(
                        wt[:],
                        src_t[lohi * C:(lohi + 1) * C, :].rearrange(
                            "(k p) c -> p k c", p=P))
                    wparts[nm] = wt
                for k in range(4):
                    nc.vector.tensor_tensor(out=Wc["r"][:, k, 0:C],
                                            in0=wparts["wr1"][:, k, :],
                                            in1=wparts["wr2"][:, k, :], op=OP.add)
                    nc.vector.tensor_copy(Wc["r"][:, k, C:2 * C],
                                          wparts["wi2"][:, k, :])
                    nc.vector.tensor_copy(Wc["q"][:, k, 0:C],
                                          wparts["wr2"][:, k, :])
                    nc.vector.tensor_tensor(out=Wc["q"][:, k, C:2 * C],
                                            in0=wparts["wi1"][:, k, :],
                                            in1=wparts["wi2"][:, k, :], op=OP.add)
                brow = s4.tile([1, C], f32, tag="brow", name="brow")
                for nm, src_b, blk in (("r", brgb_in, 0), ("q", bir_in, 1)):
                    nc.sync.dma_start(brow[:], src_b[:])
                    psb2 = ps_c_p.tile([P, C], f32, space="PSUM", tag="psb2",
                                       name="psb2")
                    nc.tensor.matmul(psb2[:], ones_row[:], brow[:],
                                     start=True, stop=True)
                    nc.vector.tensor_copy(bias[nm][:, blk * C:(blk + 1) * C],
                                          psb2[:])
                    nc.vector.memset(bias[nm][:, (1 - blk) * C:(2 - blk) * C], 0.0)
                nc.sync.dma_start(
                    wse1_sb[:],
                    wse1_in[:].rearrange("(k p) n -> p k n", p=P))
                nc.sync.dma_start(bse1_sb[:], bse1_in[:].rearrange("a b -> b a"))
                nc.sync.dma_start(wse2_sb[:], wse2_in[:])
                nc.sync.dma_start(
                    bse2_sb[:],
                    bse2_in[:].rearrange("one (c p) -> (one p) c", p=P))
                for gi, gsrc in ((1, g1_in), (2, g2_in)):
                    grow = s4.tile([1, 1], f32, tag="grow", name="grow")
                    nc.sync.dma_start(grow[:], gsrc[:])
                    psg2 = ps_c_p.tile([P, 1], f32, space="PSUM", tag="psg2",
                                       name="psg2")
                    nc.tensor.matmul(psg2[:], ones_row[:], grow[:],
                                     start=True, stop=True)
                    nc.vector.tensor_copy(gb[gi][:], psg2[:])

            # ---------------- stage 1 (per modality) ----------------
            drow_d_map = {}
            it0_ctx = ExitStack()
            ps_it0 = it0_ctx.enter_context(
                tc.tile_pool(name="psit0", bufs=1, space="PSUM"))
            s1_ctx = ExitStack()
            s1 = s1_ctx.enter_context(tc.tile_pool(name="s1", bufs=1))
            ps_ss_p = s1_ctx.enter_context(
                tc.tile_pool(name="ps_ss", bufs=1, space="PSUM"))

            def stage1_mod(m):
                ps_ss = [ps_ss_p.tile([1, C], f32, space="PSUM",
                                      tag=f"ss{h}", name=f"ss{m}{h}")
                         for h in range(2)]
                for cc in range(4):
                    raw = s1.tile([P, 64, 64], f32, tag="raw", name="raw",
                                  bufs=2)
                    nc.sync.dma_start(raw[:], mod_in[m][cc * P:(cc + 1) * P])
                    h1 = s1.tile([P, 32, 64], f32, tag="h1", name="h1",
                                 bufs=1)
                    nc.vector.tensor_tensor(out=h1[:], in0=raw[:, 0::2, :],
                                            in1=raw[:, 1::2, :], op=OP.max)
                    pf = s1.tile([P, 32, 32], f32, tag="pf", name="pf",
                                 bufs=2)
                    nc.vector.tensor_tensor(out=pf[:], in0=h1[:, :, 0::2],
                                            in1=h1[:, :, 1::2], op=OP.max)
                    pff = pf.rearrange("p a b -> p (a b)")
                    nc.scalar.activation(xb[m][cc][:], pff, AF.Copy)
                    nc.vector.tensor_copy(phalf[m][cc][:], pff[:, 0:HN])
                    sq = s1.tile([P, NT], bf16, tag="sq", name="sq", bufs=2)
                    nc.scalar.activation(sq[:], pff, AF.Square)
                    for h in range(2):
                        nc.tensor.matmul(ps_ss[h][:], ones_bf[:],
                                         sq[:, h * C:(h + 1) * C],
                                         start=(cc == 0), stop=(cc == 3))
                srow = s1.tile([1, NT], f32, tag="srow", name="srow")
                for h in range(2):
                    nc.scalar.activation(srow[:, h * C:(h + 1) * C],
                                         ps_ss[h][:], AF.Sqrt)
                nc.vector.tensor_scalar_max(srow[:], srow[:], 1e-12)
                nc.vector.reciprocal(rn[m][:], srow[:])
                nc.vector.tensor_tensor(out=srow[:], in0=rn[m][:],
                                        in1=rn[m][:], op=OP.mult)
                for h in range(2):
                    nc.vector.tensor_tensor(out=srow[:, h * C:(h + 1) * C],
                                            in0=srow[:, h * C:(h + 1) * C],
                                            in1=ps_ss[h][:], op=OP.mult)
                drow_d = dram.tile([1, NT], f32, tag=f"drow_{m}",
                                   name=f"drow_{m}")
                nc.sync.dma_start(drow_d[:], srow[:])
                drow_d_map[m] = drow_d
                rhd = dram.tile([1, HN], f32, tag=f"rhd_{m}", name=f"rhd_{m}")
                nc.sync.dma_start(rhd[:], rn[m][:, 0:HN])
                nc.sync.dma_start(
                    rni2x[m][:],
                    rhd[:].rearrange("one (c p) -> (one p) c", p=P))
                nc.vector.tensor_scalar_mul(rni2x[m][:], rni2x[m][:], 2.0)

            # ---------------- per-iteration phases ----------------
            SC_LIN = 0.505 / float(NT * KNN)
            SC_ABS = 0.495 / float(NT * KNN)
            xsrc = {"r": xb["r"], "q": xb["i"]}
            gates_a = {"r": a_r, "q": a_i}

            def open_iter_pool(it, ictx):
                return ictx.enter_context(
                    tc.tile_pool(name=f"psit{it}", bufs=1, space="PSUM"))

            def emit_table(it, tb, td, ps_it):
                use_w = Wc
                ro = 0 if tb == "r" else NT
                for i in range(8):
                    tst8 = bigp.tile([P, 2 * C], f8, tag="tst",
                                     name="tst8", bufs=3)
                    for j in range(2):
                        pst = ps_it.tile([P, C], f32, space="PSUM",
                                         tag="pst", name="pst", bufs=2)
                        for k in range(4):
                            nc.tensor.matmul(
                                pst[:],
                                xsrc[tb][k][:, i * P:(i + 1) * P],
                                use_w[tb][:, k, j * C:(j + 1) * C],
                                start=(k == 0), stop=(k == 3))
                        nc.vector.tensor_tensor(
                            out=tst8[:, j * C:(j + 1) * C], in0=pst[:],
                            in1=bias[tb][:, j * C:(j + 1) * C], op=OP.add)
                    nc.sync.dma_start(td[ro + i * P:ro + (i + 1) * P, :],
                                      tst8[:])

            def gather_phase(it, ictx, ps_it, tdram):
                dap = ictx.enter_context(
                    tc.tile_pool(name=f"dabs{it}", bufs=6))
                psS_p = ictx.enter_context(
                    tc.tile_pool(name=f"psS{it}", bufs=1, space="PSUM"))
                ps_S = {q: psS_p.tile([1, C], f32, space="PSUM", tag=f"S{q}",
                                      name=f"S{q}_{it}")
                        for q in ("lin_r", "abs_r", "lin_i", "abs_i")}
                import os as _os
                comb = _os.environ.get("COMB_GATHER", "1") == "1"
                for ch in range(NCHUNK):
                    if comb:
                        gt = bigp.tile([P, 16, 2 * C], f8, tag="big",
                                       name="gt", bufs=3)
                        nc.gpsimd.dma_gather(
                            out_ap=gt[:], in_ap=tdram[:],
                            idxs_ap=eidx3[:, ch, :],
                            num_idxs=2 * ECH, num_idxs_reg=2 * ECH,
                            elem_size=2 * C, single_packet=False)
                        g1t = gt[:, 0:8, :]
                        g2t = gt[:, 8:16, :]
                    else:
                        g1f = bigp.tile([P, 8, 2 * C], f8, tag="big",
                                        name="g1f", bufs=4)
                        nc.gpsimd.dma_gather(
                            out_ap=g1f[:], in_ap=tdram[:],
                            idxs_ap=eidx3[:, ch, 0:64],
                            num_idxs=ECH, num_idxs_reg=ECH, elem_size=2 * C)
                        g2f = bigp.tile([P, 8, 2 * C], f8, tag="big",
                                        name="g2f", bufs=4)
                        nc.gpsimd.dma_gather(
                            out_ap=g2f[:], in_ap=tdram[:],
                            idxs_ap=eidx3[:, ch, 64:128],
                            num_idxs=ECH, num_idxs_reg=ECH, elem_size=2 * C)
                        g1t = g1f[:]
                        g2t = g2f[:]
                    first = ch == 0
                    last = ch == NCHUNK - 1
                    for dirn, ga, gbuf, lo in (("r", g1t, g2t, 0),
                                               ("i", g2t, g1t, C)):
                        dd = dap.tile([P, 8, C], bf16, tag="dd", name="dd")
                        nc.vector.tensor_tensor(
                            out=dd[:], in0=ga[:, :, lo:lo + C],
                            in1=gbuf[:, :, lo:lo + C], op=OP.subtract)
                        ad = dap.tile([P, 8, C], bf16, tag="dd", name="ad")
                        nc.scalar.activation(ad[:], dd[:], AF.Abs)
                        for s in range(8):
                            nc.tensor.matmul(
                                ps_S[f"lin_{dirn}"][:], ones_bf[:], dd[:, s, :],
                                start=(first and s == 0),
                                stop=(last and s == 7))
                            nc.tensor.matmul(
                                ps_S[f"abs_{dirn}"][:], ones_bf[:], ad[:, s, :],
                                start=(first and s == 0),
                                stop=(last and s == 7))
                # S rows, AllReduce, chunked readback
                arin = dram.tile([2, C], f32, tag=f"arin{it}", name=f"arin{it}")
                arout = dram.tile([2, C], f32, tag=f"arout{it}",
                                  name=f"arout{it}")
                for row, dirn in ((0, "r"), (1, "i")):
                    t1r = dap.tile([1, C], f32, tag="t1r", name="t1r")
                    t2r = dap.tile([1, C], f32, tag="t2r", name="t2r")
                    nc.vector.tensor_scalar(t1r[:], ps_S[f"lin_{dirn}"][:],
                                            SC_LIN, None, op0=OP.mult)
                    nc.vector.tensor_scalar(t2r[:], ps_S[f"abs_{dirn}"][:],
                                            SC_ABS, None, op0=OP.mult)
                    nc.vector.tensor_tensor(out=t1r[:], in0=t1r[:], in1=t2r[:],
                                            op=OP.add)
                    nc.sync.dma_start(arin[row:row + 1, :], t1r[:])
                if timing:
                    nc.gpsimd.dma_start(arout[:], arin[:])
                else:
                    nc.gpsimd.collective_compute(
                        "AllReduce", OP.add,
                        replica_groups=[[0, 1], [2, 3], [4, 5], [6, 7]],
                        ins=[arin.opt()], outs=[arout.opt()])
                cS = dap.tile([P, 8], f32, tag="cS", name="cS")
                for row in range(2):
                    nc.sync.dma_start(
                        cS[:, row * 4:(row + 1) * 4],
                        arout[row:row + 1, :].rearrange(
                            "one (c p) -> (one p) c", p=P))
                # SE MLP
                with tc.tile_pool(name=f"ps_se{it}", bufs=1,
                                  space="PSUM") as ps_se_p:
                    ps_h1 = ps_se_p.tile([32, 1], f32, space="PSUM",
                                         tag="ps_h1", name="ps_h1")
                    for j in range(8):
                        nc.tensor.matmul(ps_h1[:], wse1_sb[:, j, :],
                                         cS[:, j:j + 1],
                                         start=(j == 0), stop=(j == 7))
                    h1r = dap.tile([32, 1], f32, tag="h1r", name="h1r")
                    nc.vector.tensor_tensor(out=h1r[:], in0=ps_h1[:],
                                            in1=bse1_sb[:], op=OP.add)
                    h1b = dap.tile([32, 1], f32, tag="h1b", name="h1b")
                    nc.vector.tensor_scalar_mul(h1b[:], h1r[:], 0.01)
                    nc.vector.tensor_tensor(out=h1r[:], in0=h1r[:], in1=h1b[:],
                                            op=OP.max)
                    ps_gate = ps_se_p.tile([P, 4], f32, space="PSUM",
                                           tag="ps_gate", name="ps_gate")
                    for j in range(4):
                        nc.tensor.matmul(ps_gate[:, j:j + 1],
                                         wse2_sb[:, j * P:(j + 1) * P],
                                         h1r[:], start=True, stop=True,
                                         skip_group_check=True)
                    gpre = dap.tile([P, 4], f32, tag="gpre", name="gpre")
                    nc.vector.tensor_tensor(out=gpre[:], in0=ps_gate[:],
                                            in1=bse2_sb[:], op=OP.add)
                    gate = dap.tile([P, 4], f32, tag="gate", name="gate")
                    nc.scalar.activation(gate[:], gpre[:], AF.Sigmoid)
                    nc.vector.tensor_tensor(out=a_r[:], in0=a_r[:], in1=gate[:],
                                            op=OP.mult)
                    omg = dap.tile([P, 4], f32, tag="omg", name="omg")
                    nc.vector.tensor_scalar(omg[:], gate[:], -1.0, 1.0,
                                            op0=OP.mult, op1=OP.add)
                    nc.vector.tensor_tensor(out=a_i[:], in0=a_i[:], in1=omg[:],
                                            op=OP.mult)
                    # fold the new gate into the combined weights in place:
                    # W_t+1 = gate_t (x) W_t along the contraction channels
                    for tb, gv in (("r", gate), ("q", omg)):
                        for k in range(4):
                            nc.vector.tensor_scalar(
                                Wc[tb][:, k, :], Wc[tb][:, k, :],
                                gv[:, k:k + 1], None, op0=OP.mult)

            # main flow: per-modality pipeline; iteration-0 tables are
            # emitted right after the modality they depend on loads, so
            # PE/DMA table work overlaps the Gram/top-k phase.
            tdram0 = dram.tile([2 * NT, 2 * C], f8, tag="Tc0", name="Tc0")
            exd_comb = dram.tile([1, 2 * E], u16, tag="exd", name="exd_comb")
            for m, tb in (("r", "r"), ("i", "q")):
                stage1_mod(m)
                emit_table(0, tb, tdram0, ps_it0)
            s1_ctx.close()
            with (
                tc.tile_pool(name="s2", bufs=2) as s2,
                tc.tile_pool(name="s2b", bufs=1) as s2b,
                tc.tile_pool(name="ps_g", bufs=2, space="PSUM") as ps_g_p,
            ):
                B = {}
                Db = {}
                for m in MODS:
                    B[m] = s2b.tile([P, NT], f32, tag=f"B{m}", name=f"B_{m}")
                    Db[m] = s2b.tile([P, NT], f32, tag=f"Db{m}", name=f"Db_{m}")
                    drow_sb = s2.tile([1, NT], f32, tag="drow_sb",
                                      name="drow_sb")
                    nc.sync.dma_start(drow_sb[:], drow_d_map[m][:])
                    for h in range(2):
                        psb = ps_g_p.tile([P, C], f32, space="PSUM", tag="psg",
                                          name="psb")
                        nc.tensor.matmul(psb[:], ones_row[:],
                                         rn[m][:, h * C:(h + 1) * C],
                                         start=True, stop=True)
                        nc.vector.tensor_copy(B[m][:, h * C:(h + 1) * C],
                                              psb[:])
                        psb = ps_g_p.tile([P, C], f32, space="PSUM", tag="psg",
                                          name="psb2")
                        nc.tensor.matmul(psb[:], ones_row[:],
                                         drow_sb[:, h * C:(h + 1) * C],
                                         start=True, stop=True)
                        nc.vector.tensor_copy(Db[m][:, h * C:(h + 1) * C],
                                              psb[:])
                for t in range(4):
                    for m in MODS:
                        moff = 0 if m == "r" else 1024
                        nd = s2.tile([P, NT], f32, tag="nd", name="nd")
                        for h in range(2):
                            psg = ps_g_p.tile([P, C], f32, space="PSUM",
                                              tag="psg", name="psg")
                            for k in range(4):
                                nc.tensor.matmul(
                                    psg[:],
                                    xb[m][k][:, t * P:(t + 1) * P],
                                    xb[m][k][:, h * C:(h + 1) * C],
                                    start=(k == 0), stop=(k == 3))
                            tmp = s2.tile([P, C], f32, tag="tmp", name="tmp")
                            nc.vector.tensor_tensor(
                                out=tmp[:], in0=psg[:],
                                in1=B[m][:, h * C:(h + 1) * C], op=OP.mult)
                            nc.vector.scalar_tensor_tensor(
                                out=nd[:, h * C:(h + 1) * C], in0=tmp[:],
                                scalar=rni2x[m][:, t:t + 1],
                                in1=Db[m][:, h * C:(h + 1) * C],
                                op0=OP.mult, op1=OP.subtract)
                        mx = s2.tile([P, 16], f32, tag="mx", name="mx")
                        nc.vector.max(out=mx[:, 0:8], in_=nd[:])
                        nc.vector.max_index(out=idx_mt[m][t][:, 0:8],
                                            in_max=mx[:, 0:8], in_values=nd[:])
                        nc.vector.match_replace(out=nd[:],
                                                in_to_replace=mx[:, 0:8],
                                                in_values=nd[:],
                                                imm_value=-1e30)
                        nc.vector.max(out=mx[:, 8:16], in_=nd[:])
                        nc.vector.max_index(out=idx_mt[m][t][:, 8:16],
                                            in_max=mx[:, 8:16], in_values=nd[:])
                        # stage the tile's edge list; modality i shifted +NT
                        # (second half of the combined gather table)
                        if m == "i":
                            sh = s2.tile([P, KNN], u16, tag="sh", name="sh")
                            nc.vector.tensor_scalar(
                                sh[:], idx_mt[m][t][:], NT, None, op0=OP.add)
                            wsrc = sh
                        else:
                            wsrc = idx_mt[m][t]
                        for hf in range(2):
                            chn = 2 * t + hf
                            base = chn * 2048 + moff
                            dst = exd_comb[0:1, base:base + 1024].rearrange(
                                "one (p k) -> (one p) k", p=64)
                            nc.sync.dma_start(
                                dst, wsrc[hf * 64:(hf + 1) * 64, :])
                    # replicated idx stripes for chunks 2t, 2t+1
                    srcidx = exd_comb[0:1, t * 4096:(t + 1) * 4096].bitcast(
                        i16).rearrange("one (c q) -> (one q) c", q=16)
                    for s8 in range(8):
                        nc.sync.dma_start(
                            eidx3[s8 * 16:(s8 + 1) * 16, 2 * t:2 * t + 2, :],
                            srcidx)

            gather_phase(0, it0_ctx, ps_it0, tdram0)
            it0_ctx.close()
            for it in range(1, iterations):
                ictx = ExitStack()
                ps_it = open_iter_pool(it, ictx)
                tdram = dram.tile([2 * NT, 2 * C], f8, tag=f"Tc{it}",
                                  name=f"Tc{it}")
                for tb in ("r", "q"):
                    emit_table(it, tb, tdram, ps_it)
                gather_phase(it, ictx, ps_it, tdram)
                ictx.close()

            # ---------------- output ----------------
            with tc.tile_pool(name="s6", bufs=2) as s6:
                alpha = s6.tile([P, 4], f32, tag="alpha", name="alpha")
                beta = s6.tile([P, 4], f32, tag="beta", name="beta")
                nc.vector.tensor_scalar(alpha[:], a_r[:], gb[1][:, 0:1], None,
                                        op0=OP.mult)
                nc.vector.tensor_scalar(beta[:], a_i[:], gb[2][:, 0:1], None,
                                        op0=OP.mult)
                for cc in range(4):
                    t1 = s6.tile([P, HN], f32, tag="t1", name="t1")
                    t2 = s6.tile([P, HN], f32, tag="t2", name="t2")
                    nc.vector.tensor_scalar(t1[:], phalf["r"][cc][:],
                                            alpha[:, cc:cc + 1], None,
                                            op0=OP.mult)
                    nc.vector.tensor_scalar(t2[:], phalf["i"][cc][:],
                                            beta[:, cc:cc + 1], None,
                                            op0=OP.mult)
                    nc.vector.tensor_tensor(out=t1[:], in0=t1[:], in1=t2[:],
                                            op=OP.add)
                    nc.vector.tensor_scalar_max(t1[:], t1[:], 0.0)
                    nc.sync.dma_start(out_t[cc * P:(cc + 1) * P, :], t1[:])

    nc.compile()
    return nc


def _prepare_in_maps(rgb, ir, W_rgb_g, b_rgb_g, W_ir_g, b_ir_g,
                     W_se1, b_se1, W_se2, b_se2, gamma1, gamma2):
    f32 = np.float32
    common = {
        "wrgb": np.ascontiguousarray(W_rgb_g, f32),
        "wir": np.ascontiguousarray(W_ir_g, f32),
        "brgb": np.ascontiguousarray(b_rgb_g, f32).reshape(1, C),
        "bir": np.ascontiguousarray(b_ir_g, f32).reshape(1, C),
        "wse1": np.ascontiguousarray(W_se1, f32),
        "bse1": np.ascontiguousarray(b_se1, f32).reshape(1, 32),
        "wse2": np.ascontiguousarray(W_se2, f32),
        "bse2": np.ascontiguousarray(b_se2, f32).reshape(1, C),
        "g1": np.asarray(gamma1, f32).reshape(1, 1),
        "g2": np.asarray(gamma2, f32).reshape(1, 1),
    }
    in_maps = []
    for core in range(N_CORES):
        s, hh = core // 2, core % 2
        r = np.asarray(rgb[s], f32)
        i = np.asarray(ir[s], f32)
        if hh:
            r = np.roll(r, -32, axis=1)
            i = np.roll(i, -32, axis=1)
        m = dict(common)
        m["rgb"] = np.ascontiguousarray(r)
        m["ir"] = np.ascontiguousarray(i)
        in_maps.append(m)
    return in_maps


def _make_runner(nc):
    """Cached replica of bass2jax.run_bass_via_pjrt's multi-core branch so
    repeated kernel() calls skip jit retracing."""
    import jax
    import concourse.mybir as mybir
    from concourse import bass2jax as b2j
    from jax.experimental.shard_map import shard_map
    from jax.sharding import Mesh, PartitionSpec

    b2j.install_neuronx_cc_hook()

    partition_name = (nc.partition_id_tensor.name
                      if nc.partition_id_tensor else None)
    in_names, out_names, out_avals, zero_outs = [], [], [], []
    for alloc in nc.m.functions[0].allocations:
        if not isinstance(alloc, mybir.MemoryLocationSet):
            continue
        name = alloc.memorylocations[0].name
        if alloc.kind == "ExternalInput":
            if name != partition_name:
                in_names.append(name)
        elif alloc.kind == "ExternalOutput":
            shape = tuple(alloc.tensor_shape)
            np_dt = mybir.dt.np(alloc.dtype)
            out_names.append(name)
            out_avals.append(jax.core.ShapedArray(shape, np_dt))
            zero_outs.append(np.zeros(shape, np_dt))

    n_params = len(in_names)
    n_outs = len(out_names)
    all_in_names = list(in_names) + list(out_names)
    if partition_name is not None:
        all_in_names.append(partition_name)
    donate = tuple(range(n_params, n_params + n_outs))

    def _body(*args):
        operands = list(args)
        if partition_name is not None:
            operands.append(b2j.partition_id_tensor())
        outs = b2j._bass_exec_p.bind(
            *operands,
            out_avals=tuple(out_avals),
            in_names=tuple(all_in_names),
            out_names=tuple(out_names),
            lowering_input_output_aliases=(),
            sim_require_finite=True,
            sim_require_nnan=True,
            nc=nc,
        )
        return tuple(outs)

    devices = jax.devices()[:N_CORES]
    mesh = Mesh(np.asarray(devices), ("core",))
    in_specs = (PartitionSpec("core"),) * (n_params + n_outs)
    out_specs = (PartitionSpec("core"),) * n_outs
    sharded = jax.jit(
        shard_map(_body, mesh=mesh, in_specs=in_specs, out_specs=out_specs,
                  check_rep=False),
        donate_argnums=donate, keep_unused=True)
    concat_zeros = [np.zeros((N_CORES * z.shape[0], *z.shape[1:]), z.dtype)
                    for z in zero_outs]

    def run(in_maps):
        concat_in = [
            np.concatenate([np.asarray(in_maps[c][nm])
                            for c in range(N_CORES)], axis=0)
            for nm in in_names
        ]
        out_arrs = sharded(*concat_in, *[z.copy() for z in concat_zeros])
        return [
            {nm: np.asarray(out_arrs[i]).reshape(
                N_CORES, *out_avals[i].shape)[c]
             for i, nm in enumerate(out_names)}
            for c in range(N_CORES)
        ]

    return run


def kernel(rgb, ir, W_rgb_g, b_rgb_g, W_ir_g, b_ir_g,
           W_se1, b_se1, W_se2, b_se2, gamma1, gamma2,
           gnn_iterations, k):
    iterations = int(gnn_iterations)
    assert int(k) == KNN, f"kernel hardcodes k=16, got {k}"
    if iterations not in _CACHE:
        import os
        nc = _build(iterations, gdt=os.environ.get("GATHER_DTYPE", "f8"))
        _CACHE[iterations] = _make_runner(nc)
    run = _CACHE[iterations]

    in_maps = _prepare_in_maps(rgb, ir, W_rgb_g, b_rgb_g, W_ir_g, b_ir_g,
                               W_se1, b_se1, W_se2, b_se2, gamma1, gamma2)
    results = run(in_maps)

    out = np.empty((4, C, 32, 32), np.float32)
    for s in range(4):
        lo = results[2 * s]["out"].reshape(C, 16, 32)
        hi = results[2 * s + 1]["out"].reshape(C, 16, 32)
        out[s] = np.concatenate([lo, hi], axis=1)
    return out
